# revision 10
# baseline (speedup 1.0000x reference)
"""Trainium2 Bass kernel for nn_ClassDiagramGNN: 2-layer GAT on 50k nodes / 850k edges.

v2 design (8 NeuronCores, dst-sharded graph parallel, bf16 + fp8 one-hots):
  - Host: add self-loops, LPT-balance dst nodes into 128-blocks per core,
    physical node layout chunked [2, cores, rows] so the h-table AllGather can
    be split into 2 overlapping collectives; bucket edges by (core, block,
    src-chunk) padded to 128-edge tiles; precompute the scatter one-hots
    S (edge->dst) and ST (dst->edge) in fp8_e4m3 on the host.
  - Phase A: h1 = x @ rhs1 in bf16 (rhs1 = [W1 | W1·a_src | W1·a_dst]),
    1280B bf16 rows [h1+b1 (512) | fp32 asrc (4)], adst kept on-chip as
    bf16 hi/lo pairs. AllGather x2 chunks overlap compute.
  - Edge pass per dst block: dma_gather bf16 source rows; per-tile matmuls
    ae = ST^T @ adst_hilo (fp8 x bf16); block-wide DVE ops for logits
    (bitcast fp32 asrc view), leaky-relu, exp; w = p (x) h via one broadcast
    (0-stride) DVE multiply; aggregation oacc/dacc by S^T matmuls in bf16;
    softmax denominator applied post-aggregation.
  - Layer 2: 256B rows (h2@W2+b2); per-edge asrc2 recomputed on-chip via
    DVE multiply + segmented reduce; same edge pass, fp32 output.
"""
import sys

for _p in ("/opt/trn_rl_repo",):
    if _p not in sys.path:
        sys.path.append(_p)

import heapq
import numpy as np
import ml_dtypes

import concourse.bass as bass
import concourse.bacc as bacc
import concourse.tile as tile
from concourse import mybir
from concourse import bass_utils

F32 = mybir.dt.float32
BF16 = mybir.dt.bfloat16
FP8 = mybir.dt.float8e4
I16 = mybir.dt.int16
AF = mybir.ActivationFunctionType
OP = mybir.AluOpType
BF16NP = ml_dtypes.bfloat16
FP8NP = ml_dtypes.float8_e4m3

# problem constants (hardcoded per contract)
N, F_IN, HID, H1, E = 50000, 512, 128, 4, 800000
NEG = 0.2
C = 8                 # cores
NS = N // C           # 6250 nodes per shard
NBLK = (NS + 127) // 128   # 49 blocks per core
CAPS = [128] * (NBLK - 1) + [NS - 128 * (NBLK - 1)]  # 48x128 + 106
CH0B = 24             # blocks in allgather chunk 0
CH0 = CH0B * 128      # 3072 rows/core in chunk 0
CH1 = NS - CH0        # 3178 rows/core in chunk 1
HALF = C * CH0        # 24576: phys row where chunk 1 starts (gather halves)
ROW1 = 512            # layer-1 row: 512 bf16 h (1024B); asrc computed on-chip
ROW2 = 128            # layer-2 row: 128 bf16 h (256B)
EPS = 1e-16

_cache = {}


def _reconfigure(n, e):
    """Testing hook: shrink the graph (keeps F_IN/HID/H1 fixed)."""
    global N, E, NS, NBLK, CAPS, CH0B, CH0, CH1, HALF
    N, E = n, e
    NS = N // C
    NBLK = (NS + 127) // 128
    CAPS = [128] * (NBLK - 1) + [NS - 128 * (NBLK - 1)]
    CH0B = NBLK // 2
    CH0 = CH0B * 128
    CH1 = NS - CH0
    HALF = C * CH0
    _cache.clear()


# --------------------------------------------------------------------------
# host-side preprocessing
# --------------------------------------------------------------------------

def _prepare(x, edge_index, W1, a_src1, a_dst1, b1, W2, a_src2, a_dst2, b2):
    # self-loop edges are handled by a local (gather-free) path on device
    src = edge_index[0].astype(np.int64)
    dst = edge_index[1].astype(np.int64)
    deg = np.bincount(dst, minlength=N) + 1

    # per-core LPT balance of dst nodes into blocks (by degree)
    local_r = np.empty(N, dtype=np.int64)     # orig id -> local row in its core
    core_of = np.empty(N, dtype=np.int64)
    perm_order = np.empty((C, NS), dtype=np.int64)  # (core, local r) -> orig id
    for c in range(C):
        ids = np.arange(c * NS, (c + 1) * NS)
        d = deg[ids]
        order = np.argsort(-d, kind="stable")
        heap = [(0, 0, i) for i in range(NBLK)]
        heapq.heapify(heap)
        assign = [[] for _ in range(NBLK)]
        for lid in order:
            while True:
                load, used, bi = heapq.heappop(heap)
                if used < CAPS[bi]:
                    break
            assign[bi].append(lid)
            heapq.heappush(heap, (load + int(d[lid]), used + 1, bi))
        pos = 0
        for bi in range(NBLK):
            for lid in assign[bi]:
                oid = c * NS + lid
                local_r[oid] = pos
                core_of[oid] = c
                perm_order[c, pos] = oid
                pos += 1

    # physical (chunked) row layout for the gather tables
    r = local_r
    phys = np.where(r < CH0, core_of * CH0 + r,
                    HALF + core_of * CH1 + (r - CH0))

    src_p = phys[src]
    dcore = core_of[dst]
    dloc = local_r[dst]
    blk = dloc // 128
    dcol = dloc % 128
    halfv = (src_p >= HALF).astype(np.int64)
    key = (dcore * NBLK + blk) * 2 + halfv
    eorder = np.argsort(key, kind="stable")
    counts = np.bincount(key, minlength=C * NBLK * 2).reshape(C, NBLK, 2)

    T_lo = -(-counts[:, :, 0].max(axis=0) // 128)  # ceil, uniform across cores
    T_hi = -(-counts[:, :, 1].max(axis=0) // 128)
    T_all = T_lo + T_hi
    TT = int(T_all.sum())
    toff = np.zeros(NBLK, np.int64)
    toff[1:] = np.cumsum(T_all)[:-1]

    src_sorted = src_p[eorder]
    dcol_sorted = dcol[eorder]
    starts = np.zeros(C * NBLK * 2 + 1, np.int64)
    starts[1:] = np.cumsum(counts.reshape(-1))

    idx_all = np.zeros((C, TT * 128), np.int16)           # pad -> row 0
    dc_all = np.full((C, TT * 128), -1, np.int64)         # pad -> -1
    for c in range(C):
        for b in range(NBLK):
            for h in range(2):
                k = (c * NBLK + b) * 2 + h
                s0, s1 = starts[k], starts[k + 1]
                n = s1 - s0
                if n == 0:
                    continue
                slot0 = (toff[b] + (T_lo[b] if h else 0)) * 128
                seg = src_sorted[s0:s1]
                if h:
                    seg = seg - HALF
                idx_all[c, slot0:slot0 + n] = seg.astype(np.int16)
                dc_all[c, slot0:slot0 + n] = dcol_sorted[s0:s1]

    # one-hot scatter matrices in fp8 (exact 0.0 / 1.0)
    slots = np.arange(TT * 128)
    t_of = slots // 128
    ep_of = slots % 128
    ONE8 = np.float32(1.0).astype(FP8NP).view(np.uint8)  # 0x38

    # weights: fold per-head attention projections into the linear transforms
    W1_64 = np.asarray(W1, np.float64)
    Dsrc1 = np.zeros((H1 * HID, H1), np.float64)
    Ddst1 = np.zeros((H1 * HID, H1), np.float64)
    for h in range(H1):
        Dsrc1[h * HID:(h + 1) * HID, h] = np.asarray(a_src1, np.float64)[h]
        Ddst1[h * HID:(h + 1) * HID, h] = np.asarray(a_dst1, np.float64)[h]
    rhs1 = np.concatenate(
        [W1_64, W1_64 @ Dsrc1, W1_64 @ Ddst1], axis=1).astype(BF16NP)  # [512,520]
    W2_64 = np.asarray(W2, np.float64)
    rhs2 = np.concatenate(
        [W2_64, W2_64 @ np.asarray(a_dst2, np.float64)[0][:, None]],
        axis=1).astype(BF16NP)                                         # [512,129]

    b1_bc = np.tile(np.asarray(b1, np.float32)[None, :], (128, 1)).astype(BF16NP)
    asrc1_bc = np.tile(np.asarray(a_src1, np.float32).reshape(1, H1 * HID),
                       (128, 1)).astype(BF16NP)
    b2_bc = np.tile(np.asarray(b2, np.float32)[None, :], (128, 1)).astype(BF16NP)
    asrc2_bc = np.tile(np.asarray(a_src2, np.float32)[0][None, :],
                       (128, 1)).astype(BF16NP)
    c2 = float(np.asarray(b2, np.float64) @ np.asarray(a_src2, np.float64)[0])
    ident = np.eye(128, dtype=np.float32)

    xnp = np.asarray(x, np.float32)
    in_maps = []
    for c in range(C):
        xT = np.ascontiguousarray(xnp[perm_order[c]].T).astype(BF16NP)  # [512, NS]
        idx_w = np.ascontiguousarray(
            np.tile(idx_all[c].reshape(-1, 16).T, (8, 1)))  # [128, TT*8]
        dc = dc_all[c]
        valid = dc >= 0
        S8 = np.zeros((128, TT * 128), np.uint8)
        S8[ep_of[valid], t_of[valid] * 128 + dc[valid]] = ONE8
        ST8 = np.zeros((128, TT * 128), np.uint8)
        ST8[dc[valid], t_of[valid] * 128 + ep_of[valid]] = ONE8
        in_maps.append({
            "xT": xT, "rhs1": rhs1, "rhs2": rhs2,
            "b1bc": b1_bc, "b2bc": b2_bc, "asrc2bc": asrc2_bc,
            "asrc1bc": asrc1_bc,
            "identf": ident,
            "idx": idx_w,
            "S8": S8.view(FP8NP), "ST8": ST8.view(FP8NP),
        })

    meta = {
        "T_lo": [int(v) for v in T_lo],
        "T_hi": [int(v) for v in T_hi],
        "toff": [int(v) for v in toff],
        "TT": TT,
        "c2": c2,
    }
    return in_maps, meta, perm_order


# --------------------------------------------------------------------------
# device program
# --------------------------------------------------------------------------

def _build(meta):
    nc = bacc.Bacc("TRN2", target_bir_lowering=False, debug=False, num_devices=C)
    TT = meta["TT"]
    c2 = meta["c2"]

    xT_d = nc.dram_tensor("xT", [F_IN, NS], BF16, kind="ExternalInput").ap()
    rhs1_d = nc.dram_tensor("rhs1", [F_IN, 520], BF16, kind="ExternalInput").ap()
    rhs2_d = nc.dram_tensor("rhs2", [F_IN, 129], BF16, kind="ExternalInput").ap()
    b1bc_d = nc.dram_tensor("b1bc", [128, 512], BF16, kind="ExternalInput").ap()
    b2bc_d = nc.dram_tensor("b2bc", [128, 128], BF16, kind="ExternalInput").ap()
    asrc2bc_d = nc.dram_tensor("asrc2bc", [128, 128], BF16, kind="ExternalInput").ap()
    asrc1bc_d = nc.dram_tensor("asrc1bc", [128, 512], BF16, kind="ExternalInput").ap()
    identf_d = nc.dram_tensor("identf", [128, 128], F32, kind="ExternalInput").ap()
    idx_d = nc.dram_tensor("idx", [128, TT * 8], I16, kind="ExternalInput").ap()
    S8_d = nc.dram_tensor("S8", [128, TT * 128], FP8, kind="ExternalInput").ap()
    ST8_d = nc.dram_tensor("ST8", [128, TT * 128], FP8, kind="ExternalInput").ap()
    out_d = nc.dram_tensor("out", [NS, HID], F32, kind="ExternalOutput").ap()

    groups = [list(range(C))]

    with tile.TileContext(nc, num_cores=C) as tc:
        with tc.tile_pool(name="dram", bufs=1, space="DRAM") as dram:
            hb1 = dram.tile([NS, ROW1], BF16)
            hfull1a = dram.tile([HALF, ROW1], BF16, addr_space="Shared")
            hfull1b = dram.tile([N - HALF, ROW1], BF16, addr_space="Shared")
            hb2 = dram.tile([NS, ROW2], BF16)
            hfull2a = dram.tile([HALF, ROW2], BF16, addr_space="Shared")
            hfull2b = dram.tile([N - HALF, ROW2], BF16, addr_space="Shared")

            with tc.tile_pool(name="lv", bufs=1) as lv:
                # long-lived constants + per-node attn scalars
                rhs1_sb = []
                rhs2_sb = []
                for k in range(4):
                    rt = lv.tile([128, 520], BF16, name=f"rhs1sb{k}")
                    nc.sync.dma_start(rt[:], rhs1_d[k * 128:(k + 1) * 128, :])
                    rhs1_sb.append(rt)
                    rt2 = lv.tile([128, 129], BF16, name=f"rhs2sb{k}")
                    nc.sync.dma_start(rt2[:], rhs2_d[k * 128:(k + 1) * 128, :])
                    rhs2_sb.append(rt2)
                b1bc_sb = lv.tile([128, 512], BF16, name="b1bc")
                nc.sync.dma_start(b1bc_sb[:], b1bc_d)
                b2bc_sb = lv.tile([128, 128], BF16, name="b2bc")
                nc.sync.dma_start(b2bc_sb[:], b2bc_d)
                asrc2bc_sb = lv.tile([128, 128], BF16, name="asrc2bc")
                nc.sync.dma_start(asrc2bc_sb[:], asrc2bc_d)
                asrc1bc_sb = lv.tile([128, 512], BF16, name="asrc1bc")
                nc.sync.dma_start(asrc1bc_sb[:], asrc1bc_d)
                identf_sb = lv.tile([128, 128], F32, name="identf")
                nc.sync.dma_start(identf_sb[:], identf_d)
                adst1_all = lv.tile([128, NBLK * 8], BF16, name="adst1")
                adst2_all = lv.tile([128, NBLK * 2], BF16, name="adst2")
                pself1_all = lv.tile([128, NBLK * 4], F32, name="pself1")
                pself2_all = lv.tile([128, NBLK], F32, name="pself2")

                # ---------------- phase A: h1 shard + attn scalars ------------
                with (
                    tc.tile_pool(name="a_w", bufs=3) as sbw,
                    tc.tile_pool(name="a_p", bufs=2, space="PSUM") as psp,
                ):
                    for b in range(NBLK):
                        bs = CAPS[b]
                        base = b * 128
                        ph = psp.tile([128, 512], F32, tag="ph")
                        pa = psp.tile([128, 8], F32, tag="pa")
                        for k in range(4):
                            xt = sbw.tile([128, 128], BF16, tag="xt")
                            nc.sync.dma_start(
                                xt[:, :bs], xT_d[k * 128:(k + 1) * 128, base:base + bs])
                            nc.tensor.matmul(ph[:bs, :], xt[:, :bs],
                                             rhs1_sb[k][:, 0:512],
                                             start=(k == 0), stop=(k == 3))
                            nc.tensor.matmul(pa[:bs, :], xt[:, :bs],
                                             rhs1_sb[k][:, 512:520],
                                             start=(k == 0), stop=(k == 3))
                        ha = sbw.tile([128, 512], BF16, tag="ha")
                        nc.vector.tensor_tensor(ha[:bs, :], ph[:bs, :],
                                                b1bc_sb[:bs, :], OP.add)
                        adh = adst1_all[:, b * 8:b * 8 + 4]
                        adl = adst1_all[:, b * 8 + 4:b * 8 + 8]
                        if bs < 128:
                            nc.vector.memset(adst1_all[:, b * 8:(b + 1) * 8], 0.0)
                        nc.scalar.activation(adh[:bs], pa[:bs, 4:8], AF.Copy)
                        nc.vector.tensor_tensor(adl[:bs], pa[:bs, 4:8], adh[:bs],
                                                OP.subtract)
                        # self-loop logits: p = exp(leaky(asrc_i + adst_i))
                        asx = sbw.tile([128, 4], F32, tag="asx")
                        nc.scalar.activation(asx[:bs], pa[:bs, 0:4], AF.Copy)
                        eps_ = sbw.tile([128, 4], F32, tag="eps_")
                        nc.vector.tensor_tensor(eps_[:bs], asx[:bs],
                                                pa[:bs, 4:8], OP.add)
                        lrs = sbw.tile([128, 4], F32, tag="lrs")
                        nc.vector.scalar_tensor_tensor(lrs[:bs], eps_[:bs], NEG,
                                                       eps_[:bs], OP.mult, OP.max)
                        if bs < 128:
                            nc.vector.memset(pself1_all[:, b * 4:(b + 1) * 4], 0.0)
                        nc.scalar.activation(pself1_all[:bs, b * 4:(b + 1) * 4],
                                             lrs[:bs], AF.Exp)
                        nc.sync.dma_start(hb1[base:base + bs, :], ha[:bs, :])
                        if b == CH0B - 1:
                            nc.gpsimd.collective_compute(
                                "AllGather", OP.bypass, replica_groups=groups,
                                ins=[hb1[0:CH0, :].opt()],
                                outs=[hfull1a[:].opt()])
                nc.gpsimd.collective_compute(
                    "AllGather", OP.bypass, replica_groups=groups,
                    ins=[hb1[CH0:NS, :].opt()], outs=[hfull1b[:].opt()])

                # ---------------- phase B: layer-1 edge pass + h2@W2 ----------
                with (
                    tc.tile_pool(name="b_m", bufs=2) as sbm,
                    tc.tile_pool(name="b_g", bufs=2) as sbg,
                    tc.tile_pool(name="b_s", bufs=2) as sbs,
                    tc.tile_pool(name="b_w", bufs=2) as sbw2,
                    tc.tile_pool(name="b_pb", bufs=2, space="PSUM") as psb,
                    tc.tile_pool(name="b_pm", bufs=2, space="PSUM") as psm,
                    tc.tile_pool(name="b_ph", bufs=1, space="PSUM") as psh,
                ):
                    for b in range(NBLK):
                        bs = CAPS[b]
                        base = b * 128
                        T_lo, T_hi = meta["T_lo"][b], meta["T_hi"][b]
                        T = T_lo + T_hi
                        boff = meta["toff"][b]

                        idx_sb = sbm.tile([128, T * 8], I16, tag="idx")
                        nc.sync.dma_start(idx_sb[:], idx_d[:, boff * 8:(boff + T) * 8])
                        S8_sb = sbm.tile([128, T * 128], FP8, tag="S8")
                        nc.sync.dma_start(S8_sb[:], S8_d[:, boff * 128:(boff + T) * 128])
                        ST8_sb = sbm.tile([128, T * 128], FP8, tag="ST8")
                        nc.sync.dma_start(ST8_sb[:], ST8_d[:, boff * 128:(boff + T) * 128])

                        gat = sbg.tile([128, T, ROW1], BF16, tag="gat")
                        if T_lo:
                            nc.gpsimd.dma_gather(
                                gat[:, 0:T_lo, :], hfull1a[:],
                                idx_sb[:, 0:T_lo * 8], T_lo * 128, T_lo * 128,
                                ROW1, elem_step=ROW1, single_packet=False)
                        if T_hi:
                            nc.gpsimd.dma_gather(
                                gat[:, T_lo:T, :], hfull1b[:],
                                idx_sb[:, T_lo * 8:T * 8], T_hi * 128, T_hi * 128,
                                ROW1, elem_step=ROW1, single_packet=False)

                        ae = psm.tile([128, T * 8], F32, tag="ae")
                        adst_blk = adst1_all[:, b * 8:(b + 1) * 8]
                        for t in range(T):
                            nc.tensor.matmul(ae[:, t * 8:(t + 1) * 8],
                                             ST8_sb[:, t * 128:(t + 1) * 128],
                                             adst_blk, start=True, stop=True)

                        tmp1 = sbg.tile([128, T * 512], BF16, tag="tmp1")
                        nc.vector.tensor_tensor(
                            tmp1[:].rearrange("p (t f) -> p t f", t=T),
                            gat[:],
                            asrc1bc_sb[:].unsqueeze(1).broadcast_to([128, T, 512]),
                            OP.mult)
                        asr = sbs.tile([128, T * 4], F32, tag="asr")
                        nc.vector.tensor_reduce(
                            asr[:].rearrange("p (t h) -> p t h", h=4),
                            tmp1[:].rearrange("p (t h f) -> p t h f", t=T, h=4),
                            mybir.AxisListType.X, OP.add)
                        ae3 = ae[:].rearrange("p (t x) -> p t x", x=8)
                        ep1 = sbs.tile([128, T * 4], F32, tag="ep1")
                        nc.vector.tensor_tensor(
                            ep1[:].rearrange("p (t h) -> p t h", h=4),
                            asr[:].rearrange("p (t h) -> p t h", h=4),
                            ae3[:, :, 0:4], OP.add)
                        ep2 = sbs.tile([128, T * 4], F32, tag="ep2")
                        nc.vector.tensor_tensor(
                            ep2[:].rearrange("p (t h) -> p t h", h=4),
                            ep1[:].rearrange("p (t h) -> p t h", h=4),
                            ae3[:, :, 4:8], OP.add)
                        lr = sbs.tile([128, T * 4], F32, tag="lr")
                        nc.vector.scalar_tensor_tensor(lr[:], ep2[:], NEG, ep2[:],
                                                       OP.mult, OP.max)
                        p_all = sbs.tile([128, T * 4], BF16, tag="p")
                        nc.scalar.activation(p_all[:], lr[:], AF.Exp)

                        w_all = sbg.tile([128, T * 512], BF16, tag="w")
                        nc.vector.tensor_tensor(
                            w_all[:].rearrange("p (t h f) -> p t h f", t=T, h=4),
                            gat[:, :, 0:512].rearrange("p t (h f) -> p t h f", h=4),
                            p_all[:].rearrange("p (t h) -> p t h", t=T)
                                .unsqueeze(3).broadcast_to([128, T, 4, 128]),
                            OP.mult)

                        oacc = psb.tile([128, 512], F32, tag="oacc")
                        dacc = psb.tile([128, 4], F32, tag="dacc")
                        for t in range(T):
                            nc.tensor.matmul(oacc[:], S8_sb[:, t * 128:(t + 1) * 128],
                                             w_all[:, t * 512:(t + 1) * 512],
                                             start=(t == 0), stop=(t == T - 1))
                            nc.tensor.matmul(dacc[:], S8_sb[:, t * 128:(t + 1) * 128],
                                             p_all[:, t * 4:(t + 1) * 4],
                                             start=(t == 0), stop=(t == T - 1))

                        # block writer: self-loop add, normalize, ELU, h2 @ rhs2
                        hself = sbw2.tile([128, 512], BF16, tag="hself")
                        nc.sync.dma_start(hself[:bs, :], hb1[base:base + bs, :])
                        selfw = sbw2.tile([128, 512], F32, tag="selfw")
                        nc.vector.tensor_tensor(
                            selfw[:].rearrange("p (h f) -> p h f", h=4),
                            hself[:].rearrange("p (h f) -> p h f", h=4),
                            pself1_all[:, b * 4:(b + 1) * 4]
                                .unsqueeze(2).broadcast_to([128, 4, 128]),
                            OP.mult)
                        osum = sbw2.tile([128, 512], F32, tag="osum")
                        nc.vector.tensor_tensor(osum[:], oacc[:], selfw[:], OP.add)
                        den = sbs.tile([128, 4], F32, tag="den")
                        nc.vector.scalar_tensor_tensor(
                            den[:], dacc[:], EPS,
                            pself1_all[:, b * 4:(b + 1) * 4], OP.add, OP.add)
                        rec = sbs.tile([128, 4], F32, tag="rec")
                        nc.vector.reciprocal(rec[:], den[:])
                        h2 = sbw2.tile([128, 512], F32, tag="h2")
                        nc.vector.tensor_tensor(
                            h2[:].rearrange("p (h f) -> p h f", h=4),
                            osum[:].rearrange("p (h f) -> p h f", h=4),
                            rec[:].unsqueeze(2).broadcast_to([128, 4, 128]),
                            OP.mult)
                        rl = sbw2.tile([128, 512], F32, tag="rl")
                        nc.scalar.activation(rl[:], h2[:], AF.Relu)
                        mn = sbw2.tile([128, 512], F32, tag="mn")
                        nc.vector.tensor_scalar_min(mn[:], h2[:], 0.0)
                        em = sbw2.tile([128, 512], F32, tag="em")
                        nc.scalar.activation(em[:], mn[:], AF.Exp)
                        h2f = sbw2.tile([128, 512], F32, tag="h2f")
                        nc.vector.scalar_tensor_tensor(h2f[:], em[:], -1.0, rl[:],
                                                       OP.add, OP.add)
                        hh = psh.tile([128, 129], F32, tag="hh")
                        for k in range(4):
                            tp = psm.tile([128, 128], F32, tag="tp", bufs=1)
                            nc.tensor.transpose(tp[:], h2f[:, k * 128:(k + 1) * 128],
                                                identf_sb[:])
                            h2T = sbs.tile([128, 128], BF16, tag="h2T")
                            nc.scalar.activation(h2T[:], tp[:], AF.Copy)
                            nc.tensor.matmul(hh[:], h2T[:], rhs2_sb[k][:],
                                             start=(k == 0), stop=(k == 3))
                        ha2 = sbw2.tile([128, 128], BF16, tag="ha2")
                        nc.vector.tensor_tensor(ha2[:bs, :], hh[:bs, 0:128],
                                                b2bc_sb[:bs, :], OP.add)
                        ad2h = adst2_all[:, b * 2:b * 2 + 1]
                        ad2l = adst2_all[:, b * 2 + 1:b * 2 + 2]
                        if bs < 128:
                            nc.vector.memset(adst2_all[:, b * 2:(b + 1) * 2], 0.0)
                        nc.scalar.activation(ad2h[:bs], hh[:bs, 128:129], AF.Copy)
                        nc.vector.tensor_tensor(ad2l[:bs], hh[:bs, 128:129],
                                                ad2h[:bs], OP.subtract)
                        # self-loop p2 = exp(leaky(asrc2_i + adst2_i))
                        tt2 = sbw2.tile([128, 128], BF16, tag="tt2")
                        nc.vector.tensor_tensor(tt2[:], ha2[:], asrc2bc_sb[:],
                                                OP.mult)
                        as2 = sbw2.tile([128, 1], F32, tag="as2")
                        nc.vector.tensor_reduce(as2[:], tt2[:],
                                                mybir.AxisListType.X, OP.add)
                        ep2s = sbw2.tile([128, 1], F32, tag="ep2s")
                        nc.vector.scalar_tensor_tensor(ep2s[:bs], as2[:bs],
                                                       -meta["c2"],
                                                       hh[:bs, 128:129],
                                                       OP.add, OP.add)
                        lr2s = sbw2.tile([128, 1], F32, tag="lr2s")
                        nc.vector.scalar_tensor_tensor(lr2s[:bs], ep2s[:bs], NEG,
                                                       ep2s[:bs], OP.mult, OP.max)
                        if bs < 128:
                            nc.vector.memset(pself2_all[:, b:b + 1], 0.0)
                        nc.scalar.activation(pself2_all[:bs, b:b + 1], lr2s[:bs],
                                             AF.Exp)
                        nc.sync.dma_start(hb2[base:base + bs, :], ha2[:bs, :])
                        if b == CH0B - 1:
                            nc.gpsimd.collective_compute(
                                "AllGather", OP.bypass, replica_groups=groups,
                                ins=[hb2[0:CH0, :].opt()],
                                outs=[hfull2a[:].opt()])
                nc.gpsimd.collective_compute(
                    "AllGather", OP.bypass, replica_groups=groups,
                    ins=[hb2[CH0:NS, :].opt()], outs=[hfull2b[:].opt()])

                # ---------------- phase D: layer-2 edge pass ------------------
                with (
                    tc.tile_pool(name="d_m", bufs=2) as sbm,
                    tc.tile_pool(name="d_g", bufs=2) as sbg,
                    tc.tile_pool(name="d_s", bufs=2) as sbs,
                    tc.tile_pool(name="d_pb", bufs=2, space="PSUM") as psb,
                    tc.tile_pool(name="d_pm", bufs=2, space="PSUM") as psm,
                ):
                    for b in range(NBLK):
                        bs = CAPS[b]
                        base = b * 128
                        T_lo, T_hi = meta["T_lo"][b], meta["T_hi"][b]
                        T = T_lo + T_hi
                        boff = meta["toff"][b]

                        idx_sb = sbm.tile([128, T * 8], I16, tag="idx")
                        nc.sync.dma_start(idx_sb[:], idx_d[:, boff * 8:(boff + T) * 8])
                        S8_sb = sbm.tile([128, T * 128], FP8, tag="S8")
                        nc.sync.dma_start(S8_sb[:], S8_d[:, boff * 128:(boff + T) * 128])
                        ST8_sb = sbm.tile([128, T * 128], FP8, tag="ST8")
                        nc.sync.dma_start(ST8_sb[:], ST8_d[:, boff * 128:(boff + T) * 128])

                        gat = sbg.tile([128, T, ROW2], BF16, tag="gat")
                        if T_lo:
                            nc.gpsimd.dma_gather(
                                gat[:, 0:T_lo, :], hfull2a[:],
                                idx_sb[:, 0:T_lo * 8], T_lo * 128, T_lo * 128,
                                ROW2, elem_step=ROW2, single_packet=False)
                        if T_hi:
                            nc.gpsimd.dma_gather(
                                gat[:, T_lo:T, :], hfull2b[:],
                                idx_sb[:, T_lo * 8:T * 8], T_hi * 128, T_hi * 128,
                                ROW2, elem_step=ROW2, single_packet=False)

                        tmp = sbg.tile([128, T * 128], BF16, tag="tmp")
                        nc.vector.tensor_tensor(
                            tmp[:].rearrange("p (t f) -> p t f", t=T),
                            gat[:],
                            asrc2bc_sb[:].unsqueeze(1).broadcast_to([128, T, 128]),
                            OP.mult)
                        asr = sbs.tile([128, T], F32, tag="asr")
                        nc.vector.tensor_reduce(
                            asr[:], tmp[:].rearrange("p (t f) -> p t f", t=T),
                            mybir.AxisListType.X, OP.add)

                        ae = psm.tile([128, T * 2], F32, tag="ae")
                        adst_blk = adst2_all[:, b * 2:(b + 1) * 2]
                        for t in range(T):
                            nc.tensor.matmul(ae[:, t * 2:(t + 1) * 2],
                                             ST8_sb[:, t * 128:(t + 1) * 128],
                                             adst_blk, start=True, stop=True)

                        ae3 = ae[:].rearrange("p (t x) -> p t x", x=2)
                        ep1 = sbs.tile([128, T], F32, tag="ep1")
                        nc.vector.scalar_tensor_tensor(
                            ep1[:].unsqueeze(2), asr[:].unsqueeze(2), -meta["c2"],
                            ae3[:, :, 0:1], OP.add, OP.add)
                        ep2 = sbs.tile([128, T], F32, tag="ep2")
                        nc.vector.tensor_tensor(ep2[:].unsqueeze(2),
                                                ep1[:].unsqueeze(2),
                                                ae3[:, :, 1:2], OP.add)
                        lr = sbs.tile([128, T], F32, tag="lr")
                        nc.vector.scalar_tensor_tensor(lr[:], ep2[:], NEG, ep2[:],
                                                       OP.mult, OP.max)
                        p_all = sbs.tile([128, T], BF16, tag="p")
                        nc.scalar.activation(p_all[:], lr[:], AF.Exp)

                        w_all = sbg.tile([128, T * 128], BF16, tag="w")
                        nc.vector.tensor_tensor(
                            w_all[:].rearrange("p (t f) -> p t f", t=T),
                            gat[:],
                            p_all[:].unsqueeze(2).broadcast_to([128, T, 128]),
                            OP.mult)

                        oacc = psb.tile([128, 128], F32, tag="oacc")
                        dacc = psb.tile([128, 1], F32, tag="dacc")
                        for t in range(T):
                            nc.tensor.matmul(oacc[:], S8_sb[:, t * 128:(t + 1) * 128],
                                             w_all[:, t * 128:(t + 1) * 128],
                                             start=(t == 0), stop=(t == T - 1))
                            nc.tensor.matmul(dacc[:], S8_sb[:, t * 128:(t + 1) * 128],
                                             p_all[:, t:t + 1],
                                             start=(t == 0), stop=(t == T - 1))

                        h2self = sbs.tile([128, 128], BF16, tag="h2self")
                        nc.sync.dma_start(h2self[:bs, :], hb2[base:base + bs, :])
                        selfw2 = sbs.tile([128, 128], F32, tag="selfw2")
                        nc.vector.tensor_single_scalar(
                            selfw2[:], h2self[:], pself2_all[:, b:b + 1], OP.mult)
                        osum2 = sbs.tile([128, 128], F32, tag="osum2")
                        nc.vector.tensor_tensor(osum2[:], oacc[:], selfw2[:], OP.add)
                        den = sbs.tile([128, 1], F32, tag="den")
                        nc.vector.scalar_tensor_tensor(
                            den[:], dacc[:], EPS, pself2_all[:, b:b + 1],
                            OP.add, OP.add)
                        rec = sbs.tile([128, 1], F32, tag="rec")
                        nc.vector.reciprocal(rec[:], den[:])
                        of = sbs.tile([128, 128], F32, tag="of")
                        nc.scalar.activation(of[:], osum2[:], AF.Copy,
                                             scale=rec[:, 0:1])
                        nc.sync.dma_start(out_d[base:base + bs, :], of[:bs, :])

    nc.compile()
    return nc


# --------------------------------------------------------------------------
# entry point
# --------------------------------------------------------------------------

def kernel(x, edge_index, W1, a_src1, a_dst1, b1, W2, a_src2, a_dst2, b2,
           _trace=False):
    in_maps, meta, perm_order = _prepare(
        x, edge_index, W1, a_src1, a_dst1, b1, W2, a_src2, a_dst2, b2)

    import time as _time
    _t0 = _time.time()
    key = (meta["TT"], tuple(meta["T_lo"]), tuple(meta["T_hi"]))
    if key not in _cache:
        _cache.clear()
        _cache[key] = _build(meta)
    nc = _cache[key]
    print(f"[kernel] build done at {_time.time()-_t0:.1f}s", flush=True)

    kw = {}
    if _trace:
        kw = dict(trace=True)
    res = bass_utils.run_bass_kernel_spmd(nc, in_maps, core_ids=list(range(C)), **kw)

    out = np.empty((N, HID), np.float32)
    for c in range(C):
        out[perm_order[c]] = res.results[c]["out"]
    kernel._last_result = res
    return out


# revision 11
# speedup vs baseline: 1.6070x; 1.6070x over previous
"""Trainium2 Bass kernel for nn_ClassDiagramGNN: 2-layer GAT on 50k nodes / 850k edges.

v4 design (8 NeuronCores, dst-sharded graph parallel, bf16 + fp8 one-hots):
  - Host: LPT-balance dst nodes into 128-blocks per core; physical node layout
    chunked [2, cores, rows] so each table AllGather splits into 2 overlapping
    collectives; non-loop edges bucketed by (core, block, src-chunk) padded to
    128-edge tiles; scatter one-hots S (edge->dst) / ST (dst->edge) in fp8.
  - Self-loop edges never gathered: their softmax terms are computed from
    local per-block data and added to numerator/denominator directly.
  - Layer-1 table rows: 1280B bf16 [h1+b1 (512, feat-major (f,h) interleave) |
    fp32 asrc (4) | pad]. Feat-major layout gives the p (x) h broadcast
    multiply a packed head-minor last dim (DVE 2x eligible).
  - Edge pass per dst block: dma_gather rows; per-tile ae = ST^T @ adst_hilo
    (fp8 x bf16 matmul); block-wide DVE logits (bitcast fp32 asrc view),
    leaky-relu, exp; one broadcast DVE multiply for w = p (x) h; aggregation
    oacc/dacc via S^T matmuls accumulated in PSUM; denominator post-applied.
  - Layer 2: 256B rows (h2@W2+b2); asrc2 recomputed on-chip (multiply +
    segmented reduce); pair-duplicated p2 keeps the multiply packed.
"""
import sys

for _p in ("/opt/trn_rl_repo",):
    if _p not in sys.path:
        sys.path.append(_p)

import heapq
import numpy as np
import ml_dtypes

import concourse.bass as bass
import concourse.bacc as bacc
import concourse.tile as tile
from concourse import mybir
from concourse import bass_utils

F32 = mybir.dt.float32
BF16 = mybir.dt.bfloat16
FP8 = mybir.dt.float8e4
I16 = mybir.dt.int16
AF = mybir.ActivationFunctionType
OP = mybir.AluOpType
BF16NP = ml_dtypes.bfloat16
FP8NP = ml_dtypes.float8_e4m3

# problem constants (hardcoded per contract)
N, F_IN, HID, H1, E = 50000, 512, 128, 4, 800000
NEG = 0.2
C = 8                 # cores
NS = N // C           # 6250 nodes per shard
NBLK = (NS + 127) // 128   # 49 blocks per core
CAPS = [128] * (NBLK - 1) + [NS - 128 * (NBLK - 1)]  # 48x128 + 106
CH0B = 24             # blocks in allgather chunk 0
CH0 = CH0B * 128      # 3072 rows/core in chunk 0
CH1 = NS - CH0        # 3178 rows/core in chunk 1
HALF = C * CH0        # 24576: phys row where chunk 1 starts (gather halves)
ROW1 = 640            # layer-1 row: 512 bf16 h (feat-major) | 8 slots fp32 asrc | pad
ROW2 = 128            # layer-2 row: 128 bf16 h (256B)
EPS = 1e-16

_cache = {}


def _reconfigure(n, e):
    """Testing hook: shrink the graph (keeps F_IN/HID/H1 fixed)."""
    global N, E, NS, NBLK, CAPS, CH0B, CH0, CH1, HALF
    N, E = n, e
    NS = N // C
    NBLK = (NS + 127) // 128
    CAPS = [128] * (NBLK - 1) + [NS - 128 * (NBLK - 1)]
    CH0B = NBLK // 2
    CH0 = CH0B * 128
    CH1 = NS - CH0
    HALF = C * CH0
    _cache.clear()


# --------------------------------------------------------------------------
# host-side preprocessing
# --------------------------------------------------------------------------

def _prepare(x, edge_index, W1, a_src1, a_dst1, b1, W2, a_src2, a_dst2, b2):
    # self-loop edges are handled by a local (gather-free) path on device
    src = edge_index[0].astype(np.int64)
    dst = edge_index[1].astype(np.int64)
    deg = np.bincount(dst, minlength=N) + 1

    # per-core LPT balance of dst nodes into blocks (by degree)
    local_r = np.empty(N, dtype=np.int64)     # orig id -> local row in its core
    core_of = np.empty(N, dtype=np.int64)
    perm_order = np.empty((C, NS), dtype=np.int64)  # (core, local r) -> orig id
    for c in range(C):
        ids = np.arange(c * NS, (c + 1) * NS)
        d = deg[ids]
        order = np.argsort(-d, kind="stable")
        heap = [(0, 0, i) for i in range(NBLK)]
        heapq.heapify(heap)
        assign = [[] for _ in range(NBLK)]
        for lid in order:
            while True:
                load, used, bi = heapq.heappop(heap)
                if used < CAPS[bi]:
                    break
            assign[bi].append(lid)
            heapq.heappush(heap, (load + int(d[lid]), used + 1, bi))
        pos = 0
        for bi in range(NBLK):
            for lid in assign[bi]:
                oid = c * NS + lid
                local_r[oid] = pos
                core_of[oid] = c
                perm_order[c, pos] = oid
                pos += 1

    # physical (chunked) row layout for the gather tables
    r = local_r
    phys = np.where(r < CH0, core_of * CH0 + r,
                    HALF + core_of * CH1 + (r - CH0))

    src_p = phys[src]
    dcore = core_of[dst]
    dloc = local_r[dst]
    blk = dloc // 128
    dcol = dloc % 128
    halfv = (src_p >= HALF).astype(np.int64)
    key = (dcore * NBLK + blk) * 2 + halfv
    eorder = np.argsort(key, kind="stable")
    counts = np.bincount(key, minlength=C * NBLK * 2).reshape(C, NBLK, 2)

    T_lo = -(-counts[:, :, 0].max(axis=0) // 128)  # ceil, uniform across cores
    T_hi = -(-counts[:, :, 1].max(axis=0) // 128)
    T_all = T_lo + T_hi
    TT = int(T_all.sum())
    toff = np.zeros(NBLK, np.int64)
    toff[1:] = np.cumsum(T_all)[:-1]

    src_sorted = src_p[eorder]
    dcol_sorted = dcol[eorder]
    starts = np.zeros(C * NBLK * 2 + 1, np.int64)
    starts[1:] = np.cumsum(counts.reshape(-1))

    idx_all = np.zeros((C, TT * 128), np.int16)           # pad -> row 0
    dc_all = np.full((C, TT * 128), -1, np.int64)         # pad -> -1
    for c in range(C):
        for b in range(NBLK):
            for h in range(2):
                k = (c * NBLK + b) * 2 + h
                s0, s1 = starts[k], starts[k + 1]
                n = s1 - s0
                if n == 0:
                    continue
                slot0 = (toff[b] + (T_lo[b] if h else 0)) * 128
                seg = src_sorted[s0:s1]
                if h:
                    seg = seg - HALF
                idx_all[c, slot0:slot0 + n] = seg.astype(np.int16)
                dc_all[c, slot0:slot0 + n] = dcol_sorted[s0:s1]

    # one-hot scatter matrices in fp8 (exact 0.0 / 1.0)
    slots = np.arange(TT * 128)
    t_of = slots // 128
    ep_of = slots % 128
    ONE8 = np.float32(1.0).astype(FP8NP).view(np.uint8)  # 0x38

    # weights: fold attention projections in; feat-major (f,h) column order
    W1_64 = np.asarray(W1, np.float64)
    Dsrc1 = np.zeros((H1 * HID, H1), np.float64)
    Ddst1 = np.zeros((H1 * HID, H1), np.float64)
    for h in range(H1):
        Dsrc1[h * HID:(h + 1) * HID, h] = np.asarray(a_src1, np.float64)[h]
        Ddst1[h * HID:(h + 1) * HID, h] = np.asarray(a_dst1, np.float64)[h]
    # feat-major index: new position f*4+h  <- old position h*128+f
    ff = np.arange(512)
    fmaj = (ff % 4) * 128 + ff // 4   # old index stored at new slot ff
    rhs1 = np.concatenate(
        [W1_64[:, fmaj], W1_64 @ Dsrc1, W1_64 @ Ddst1], axis=1).astype(BF16NP)
    W2_64 = np.asarray(W2, np.float64)  # rows indexed by old h*128+f order
    rhs2 = np.concatenate(
        [W2_64[fmaj, :],
         (W2_64 @ np.asarray(a_dst2, np.float64)[0][:, None])[fmaj, :]],
        axis=1).astype(BF16NP)                            # [512 (f-major), 129]

    b1_fm = np.asarray(b1, np.float64)[fmaj]
    b1_bc = np.tile(b1_fm[None, :], (128, 1)).astype(BF16NP)
    b2_bc = np.tile(np.asarray(b2, np.float32)[None, :], (128, 1)).astype(BF16NP)
    asrc2_bc = np.tile(np.asarray(a_src2, np.float32)[0][None, :],
                       (128, 1)).astype(BF16NP)
    c2 = float(np.asarray(b2, np.float64) @ np.asarray(a_src2, np.float64)[0])
    ident = np.eye(128, dtype=np.float32)

    xnp = np.asarray(x, np.float32)
    in_maps = []
    for c in range(C):
        xT = np.ascontiguousarray(xnp[perm_order[c]].T).astype(BF16NP)  # [512, NS]
        idx_w = np.ascontiguousarray(
            np.tile(idx_all[c].reshape(-1, 16).T, (8, 1)))  # [128, TT*8]
        dc = dc_all[c]
        valid = dc >= 0
        S8 = np.zeros((128, TT * 128), np.uint8)
        S8[ep_of[valid], t_of[valid] * 128 + dc[valid]] = ONE8
        ST8 = np.zeros((128, TT * 128), np.uint8)
        ST8[dc[valid], t_of[valid] * 128 + ep_of[valid]] = ONE8
        in_maps.append({
            "xT": xT, "rhs1": rhs1, "rhs2": rhs2,
            "b1bc": b1_bc, "b2bc": b2_bc, "asrc2bc": asrc2_bc,
            "identf": ident,
            "idx": idx_w,
            "S8": S8.view(FP8NP), "ST8": ST8.view(FP8NP),
        })

    meta = {
        "T_lo": [int(v) for v in T_lo],
        "T_hi": [int(v) for v in T_hi],
        "toff": [int(v) for v in toff],
        "TT": TT,
        "c2": c2,
    }
    return in_maps, meta, perm_order


# --------------------------------------------------------------------------
# device program
# --------------------------------------------------------------------------

def _build(meta):
    nc = bacc.Bacc("TRN2", target_bir_lowering=False, debug=False, num_devices=C)
    TT = meta["TT"]

    xT_d = nc.dram_tensor("xT", [F_IN, NS], BF16, kind="ExternalInput").ap()
    rhs1_d = nc.dram_tensor("rhs1", [F_IN, 520], BF16, kind="ExternalInput").ap()
    rhs2_d = nc.dram_tensor("rhs2", [F_IN, 129], BF16, kind="ExternalInput").ap()
    b1bc_d = nc.dram_tensor("b1bc", [128, 512], BF16, kind="ExternalInput").ap()
    b2bc_d = nc.dram_tensor("b2bc", [128, 128], BF16, kind="ExternalInput").ap()
    asrc2bc_d = nc.dram_tensor("asrc2bc", [128, 128], BF16, kind="ExternalInput").ap()
    identf_d = nc.dram_tensor("identf", [128, 128], F32, kind="ExternalInput").ap()
    idx_d = nc.dram_tensor("idx", [128, TT * 8], I16, kind="ExternalInput").ap()
    S8_d = nc.dram_tensor("S8", [128, TT * 128], FP8, kind="ExternalInput").ap()
    ST8_d = nc.dram_tensor("ST8", [128, TT * 128], FP8, kind="ExternalInput").ap()
    out_d = nc.dram_tensor("out", [NS, HID], F32, kind="ExternalOutput").ap()

    groups = [list(range(C))]

    with tile.TileContext(nc, num_cores=C) as tc:
        with tc.tile_pool(name="dram", bufs=1, space="DRAM") as dram:
            hb1 = dram.tile([NS, ROW1], BF16)
            hfull1a = dram.tile([HALF, ROW1], BF16, addr_space="Shared")
            hfull1b = dram.tile([N - HALF, ROW1], BF16, addr_space="Shared")
            hb2 = dram.tile([NS, ROW2], BF16)
            hfull2a = dram.tile([HALF, ROW2], BF16, addr_space="Shared")
            hfull2b = dram.tile([N - HALF, ROW2], BF16, addr_space="Shared")

            with tc.tile_pool(name="lv", bufs=1) as lv:
                rhs1_sb = []
                rhs2_sb = []
                for k in range(4):
                    rt = lv.tile([128, 520], BF16, name=f"rhs1sb{k}")
                    nc.sync.dma_start(rt[:], rhs1_d[k * 128:(k + 1) * 128, :])
                    rhs1_sb.append(rt)
                    rt2 = lv.tile([128, 129], BF16, name=f"rhs2sb{k}")
                    nc.sync.dma_start(rt2[:], rhs2_d[k * 128:(k + 1) * 128, :])
                    rhs2_sb.append(rt2)
                b1bc_sb = lv.tile([128, 512], BF16, name="b1bc")
                nc.sync.dma_start(b1bc_sb[:], b1bc_d)
                b2bc_sb = lv.tile([128, 128], BF16, name="b2bc")
                nc.sync.dma_start(b2bc_sb[:], b2bc_d)
                asrc2bc_sb = lv.tile([128, 128], BF16, name="asrc2bc")
                nc.sync.dma_start(asrc2bc_sb[:], asrc2bc_d)
                identf_sb = lv.tile([128, 128], F32, name="identf")
                nc.sync.dma_start(identf_sb[:], identf_d)
                adst1_all = lv.tile([128, NBLK * 8], BF16, name="adst1")
                adst2_all = lv.tile([128, NBLK * 2], BF16, name="adst2")
                pself1_all = lv.tile([128, NBLK * 4], F32, name="pself1")
                pself2_all = lv.tile([128, NBLK], F32, name="pself2")

                # ---------------- phase A: h1 shard + attn scalars ------------
                with (
                    tc.tile_pool(name="a_w", bufs=3) as sbw,
                    tc.tile_pool(name="a_p", bufs=2, space="PSUM") as psp,
                ):
                    for b in range(NBLK):
                        bs = CAPS[b]
                        base = b * 128
                        ph = psp.tile([128, 512], F32, tag="ph")
                        pa = psp.tile([128, 8], F32, tag="pa")
                        for k in range(4):
                            xt = sbw.tile([128, 128], BF16, tag="xt")
                            nc.sync.dma_start(
                                xt[:, :bs], xT_d[k * 128:(k + 1) * 128, base:base + bs])
                            nc.tensor.matmul(ph[:bs, :], xt[:, :bs],
                                             rhs1_sb[k][:, 0:512],
                                             start=(k == 0), stop=(k == 3))
                            nc.tensor.matmul(pa[:bs, :], xt[:, :bs],
                                             rhs1_sb[k][:, 512:520],
                                             start=(k == 0), stop=(k == 3))
                        ha = sbw.tile([128, 512], BF16, tag="ha")
                        nc.vector.tensor_tensor(ha[:bs, :], ph[:bs, :],
                                                b1bc_sb[:bs, :], OP.add)
                        asx = sbw.tile([128, 4], F32, tag="asx")
                        nc.scalar.activation(asx[:bs], pa[:bs, 0:4], AF.Copy)
                        adh = adst1_all[:, b * 8:b * 8 + 4]
                        adl = adst1_all[:, b * 8 + 4:b * 8 + 8]
                        if bs < 128:
                            nc.vector.memset(adst1_all[:, b * 8:(b + 1) * 8], 0.0)
                        nc.scalar.activation(adh[:bs], pa[:bs, 4:8], AF.Copy)
                        nc.vector.tensor_tensor(adl[:bs], pa[:bs, 4:8], adh[:bs],
                                                OP.subtract)
                        # self-loop logits: p = exp(leaky(asrc_i + adst_i))
                        eps_ = sbw.tile([128, 4], F32, tag="eps_")
                        nc.vector.tensor_tensor(eps_[:bs], asx[:bs],
                                                pa[:bs, 4:8], OP.add)
                        lrs = sbw.tile([128, 4], F32, tag="lrs")
                        nc.vector.scalar_tensor_tensor(lrs[:bs], eps_[:bs], NEG,
                                                       eps_[:bs], OP.mult, OP.max)
                        if bs < 128:
                            nc.vector.memset(pself1_all[:, b * 4:(b + 1) * 4], 0.0)
                        nc.scalar.activation(pself1_all[:bs, b * 4:(b + 1) * 4],
                                             lrs[:bs], AF.Exp)
                        nc.sync.dma_start(hb1[base:base + bs, 0:512], ha[:bs, :])
                        nc.sync.dma_start(hb1[base:base + bs, 512:520],
                                          asx[:bs, :].bitcast(BF16))
                        if b == CH0B - 1:
                            nc.gpsimd.collective_compute(
                                "AllGather", OP.bypass, replica_groups=groups,
                                ins=[hb1[0:CH0, :].opt()],
                                outs=[hfull1a[:].opt()])
                nc.gpsimd.collective_compute(
                    "AllGather", OP.bypass, replica_groups=groups,
                    ins=[hb1[CH0:NS, :].opt()], outs=[hfull1b[:].opt()])

                # ---------------- phase B: layer-1 edge pass + h2@W2 ----------
                with (
                    tc.tile_pool(name="b_m", bufs=3) as sbm,
                    tc.tile_pool(name="b_g", bufs=2) as sbg,
                    tc.tile_pool(name="b_s", bufs=2) as sbs,
                    tc.tile_pool(name="b_w", bufs=2) as sbw2,
                    tc.tile_pool(name="b_pb", bufs=2, space="PSUM") as psb,
                    tc.tile_pool(name="b_pm", bufs=2, space="PSUM") as psm,
                    tc.tile_pool(name="b_ph", bufs=1, space="PSUM") as psh,
                ):
                    for b in range(NBLK):
                        bs = CAPS[b]
                        base = b * 128
                        T_lo, T_hi = meta["T_lo"][b], meta["T_hi"][b]
                        T = T_lo + T_hi
                        boff = meta["toff"][b]

                        idx_sb = sbm.tile([128, T * 8], I16, tag="idx")
                        nc.sync.dma_start(idx_sb[:], idx_d[:, boff * 8:(boff + T) * 8])
                        S8_sb = sbm.tile([128, T * 128], FP8, tag="S8")
                        nc.sync.dma_start(S8_sb[:], S8_d[:, boff * 128:(boff + T) * 128])
                        ST8_sb = sbm.tile([128, T * 128], FP8, tag="ST8")
                        nc.sync.dma_start(ST8_sb[:], ST8_d[:, boff * 128:(boff + T) * 128])

                        gat = sbg.tile([128, T, ROW1], BF16, tag="gat")
                        if T_lo:
                            nc.gpsimd.dma_gather(
                                gat[:, 0:T_lo, :], hfull1a[:],
                                idx_sb[:, 0:T_lo * 8], T_lo * 128, T_lo * 128,
                                ROW1, elem_step=ROW1, single_packet=False)
                        if T_hi:
                            nc.gpsimd.dma_gather(
                                gat[:, T_lo:T, :], hfull1b[:],
                                idx_sb[:, T_lo * 8:T * 8], T_hi * 128, T_hi * 128,
                                ROW1, elem_step=ROW1, single_packet=False)

                        ae = psm.tile([128, T * 8], F32, tag="ae")
                        adst_blk = adst1_all[:, b * 8:(b + 1) * 8]
                        for t in range(T):
                            nc.tensor.matmul(ae[:, t * 8:(t + 1) * 8],
                                             ST8_sb[:, t * 128:(t + 1) * 128],
                                             adst_blk, start=True, stop=True)

                        asrc_v = gat[:, :, 512:520].bitcast(F32)      # [128,T,4]
                        ae3 = ae[:].rearrange("p (t x) -> p t x", x=8)
                        ep1 = sbs.tile([128, T * 4], F32, tag="ep1")
                        nc.vector.tensor_tensor(
                            ep1[:].rearrange("p (t h) -> p t h", h=4),
                            asrc_v, ae3[:, :, 0:4], OP.add)
                        ep2 = sbs.tile([128, T * 4], F32, tag="ep2")
                        nc.vector.tensor_tensor(
                            ep2[:].rearrange("p (t h) -> p t h", h=4),
                            ep1[:].rearrange("p (t h) -> p t h", h=4),
                            ae3[:, :, 4:8], OP.add)
                        lr = sbs.tile([128, T * 4], F32, tag="lr")
                        nc.vector.scalar_tensor_tensor(lr[:], ep2[:], NEG, ep2[:],
                                                       OP.mult, OP.max)
                        p_all = sbs.tile([128, T * 4], BF16, tag="p")
                        nc.scalar.activation(p_all[:], lr[:], AF.Exp)

                        # w[e, t, f, h] = h[e, t, f, h] * p[e, t, h] (feat-major)
                        w_all = sbg.tile([128, T * 512], BF16, tag="w")
                        nc.vector.tensor_tensor(
                            w_all[:].rearrange("p (t f h) -> p t f h", t=T, h=4),
                            gat[:, :, 0:512].rearrange("p t (f h) -> p t f h", h=4),
                            p_all[:].rearrange("p (t h) -> p t h", t=T)
                                .unsqueeze(2).broadcast_to([128, T, 128, 4]),
                            OP.mult)

                        oacc = psb.tile([128, 512], F32, tag="oacc")
                        dacc = psb.tile([128, 4], F32, tag="dacc")
                        for t in range(T):
                            nc.tensor.matmul(oacc[:], S8_sb[:, t * 128:(t + 1) * 128],
                                             w_all[:, t * 512:(t + 1) * 512],
                                             start=(t == 0), stop=(t == T - 1))
                            nc.tensor.matmul(dacc[:], S8_sb[:, t * 128:(t + 1) * 128],
                                             p_all[:, t * 4:(t + 1) * 4],
                                             start=(t == 0), stop=(t == T - 1))

                        # block writer: self-loop add, normalize, ELU, h2 @ rhs2
                        hself = sbw2.tile([128, 512], BF16, tag="hself")
                        nc.sync.dma_start(hself[:bs, :], hb1[base:base + bs, 0:512])
                        selfw = sbw2.tile([128, 512], F32, tag="selfw")
                        nc.vector.tensor_tensor(
                            selfw[:].rearrange("p (f h) -> p f h", h=4),
                            hself[:].rearrange("p (f h) -> p f h", h=4),
                            pself1_all[:, b * 4:(b + 1) * 4]
                                .unsqueeze(1).broadcast_to([128, 128, 4]),
                            OP.mult)
                        osum = sbw2.tile([128, 512], F32, tag="osum")
                        nc.vector.tensor_tensor(osum[:], oacc[:], selfw[:], OP.add)
                        den = sbs.tile([128, 4], F32, tag="den")
                        nc.vector.scalar_tensor_tensor(
                            den[:], dacc[:], EPS,
                            pself1_all[:, b * 4:(b + 1) * 4], OP.add, OP.add)
                        rec = sbs.tile([128, 4], F32, tag="rec")
                        nc.vector.reciprocal(rec[:], den[:])
                        h2 = sbw2.tile([128, 512], F32, tag="h2")
                        nc.vector.tensor_tensor(
                            h2[:].rearrange("p (f h) -> p f h", h=4),
                            osum[:].rearrange("p (f h) -> p f h", h=4),
                            rec[:].unsqueeze(1).broadcast_to([128, 128, 4]),
                            OP.mult)
                        rl = sbw2.tile([128, 512], F32, tag="rl")
                        nc.scalar.activation(rl[:], h2[:], AF.Relu)
                        mn = sbw2.tile([128, 512], F32, tag="mn")
                        nc.vector.tensor_scalar_min(mn[:], h2[:], 0.0)
                        em = sbw2.tile([128, 512], F32, tag="em")
                        nc.scalar.activation(em[:], mn[:], AF.Exp)
                        h2f = sbw2.tile([128, 512], F32, tag="h2f")
                        nc.vector.scalar_tensor_tensor(h2f[:], em[:], -1.0, rl[:],
                                                       OP.add, OP.add)
                        hh = psh.tile([128, 129], F32, tag="hh")
                        for k in range(4):
                            tp = psm.tile([128, 128], F32, tag="tp", bufs=1)
                            nc.tensor.transpose(tp[:], h2f[:, k * 128:(k + 1) * 128],
                                                identf_sb[:])
                            h2T = sbs.tile([128, 128], BF16, tag="h2T")
                            nc.scalar.activation(h2T[:], tp[:], AF.Copy)
                            nc.tensor.matmul(hh[:], h2T[:], rhs2_sb[k][:],
                                             start=(k == 0), stop=(k == 3))
                        ha2 = sbw2.tile([128, 128], BF16, tag="ha2")
                        nc.vector.tensor_tensor(ha2[:bs, :], hh[:bs, 0:128],
                                                b2bc_sb[:bs, :], OP.add)
                        ad2h = adst2_all[:, b * 2:b * 2 + 1]
                        ad2l = adst2_all[:, b * 2 + 1:b * 2 + 2]
                        if bs < 128:
                            nc.vector.memset(adst2_all[:, b * 2:(b + 1) * 2], 0.0)
                        nc.scalar.activation(ad2h[:bs], hh[:bs, 128:129], AF.Copy)
                        nc.vector.tensor_tensor(ad2l[:bs], hh[:bs, 128:129],
                                                ad2h[:bs], OP.subtract)
                        # self-loop p2 = exp(leaky(asrc2_i + adst2_i))
                        tt2 = sbw2.tile([128, 128], BF16, tag="tt2")
                        nc.vector.tensor_tensor(tt2[:], ha2[:], asrc2bc_sb[:],
                                                OP.mult)
                        as2 = sbw2.tile([128, 1], F32, tag="as2")
                        nc.vector.tensor_reduce(as2[:], tt2[:],
                                                mybir.AxisListType.X, OP.add)
                        ep2s = sbw2.tile([128, 1], F32, tag="ep2s")
                        nc.vector.scalar_tensor_tensor(ep2s[:bs], as2[:bs],
                                                       -meta["c2"],
                                                       hh[:bs, 128:129],
                                                       OP.add, OP.add)
                        lr2s = sbw2.tile([128, 1], F32, tag="lr2s")
                        nc.vector.scalar_tensor_tensor(lr2s[:bs], ep2s[:bs], NEG,
                                                       ep2s[:bs], OP.mult, OP.max)
                        if bs < 128:
                            nc.vector.memset(pself2_all[:, b:b + 1], 0.0)
                        nc.scalar.activation(pself2_all[:bs, b:b + 1], lr2s[:bs],
                                             AF.Exp)
                        nc.sync.dma_start(hb2[base:base + bs, :], ha2[:bs, :])
                        if b == CH0B - 1:
                            nc.gpsimd.collective_compute(
                                "AllGather", OP.bypass, replica_groups=groups,
                                ins=[hb2[0:CH0, :].opt()],
                                outs=[hfull2a[:].opt()])
                nc.gpsimd.collective_compute(
                    "AllGather", OP.bypass, replica_groups=groups,
                    ins=[hb2[CH0:NS, :].opt()], outs=[hfull2b[:].opt()])

                # ---------------- phase D: layer-2 edge pass ------------------
                with (
                    tc.tile_pool(name="d_m", bufs=3) as sbm,
                    tc.tile_pool(name="d_g", bufs=3) as sbg,
                    tc.tile_pool(name="d_s", bufs=3) as sbs,
                    tc.tile_pool(name="d_pb", bufs=2, space="PSUM") as psb,
                    tc.tile_pool(name="d_pm", bufs=2, space="PSUM") as psm,
                ):
                    for b in range(NBLK):
                        bs = CAPS[b]
                        base = b * 128
                        T_lo, T_hi = meta["T_lo"][b], meta["T_hi"][b]
                        T = T_lo + T_hi
                        boff = meta["toff"][b]

                        idx_sb = sbm.tile([128, T * 8], I16, tag="idx")
                        nc.sync.dma_start(idx_sb[:], idx_d[:, boff * 8:(boff + T) * 8])
                        S8_sb = sbm.tile([128, T * 128], FP8, tag="S8")
                        nc.sync.dma_start(S8_sb[:], S8_d[:, boff * 128:(boff + T) * 128])
                        ST8_sb = sbm.tile([128, T * 128], FP8, tag="ST8")
                        nc.sync.dma_start(ST8_sb[:], ST8_d[:, boff * 128:(boff + T) * 128])

                        gat = sbg.tile([128, T, ROW2], BF16, tag="gat")
                        if T_lo:
                            nc.gpsimd.dma_gather(
                                gat[:, 0:T_lo, :], hfull2a[:],
                                idx_sb[:, 0:T_lo * 8], T_lo * 128, T_lo * 128,
                                ROW2, elem_step=ROW2, single_packet=False)
                        if T_hi:
                            nc.gpsimd.dma_gather(
                                gat[:, T_lo:T, :], hfull2b[:],
                                idx_sb[:, T_lo * 8:T * 8], T_hi * 128, T_hi * 128,
                                ROW2, elem_step=ROW2, single_packet=False)

                        tmp = sbg.tile([128, T * 128], BF16, tag="tmp")
                        nc.vector.tensor_tensor(
                            tmp[:].rearrange("p (t f) -> p t f", t=T),
                            gat[:],
                            asrc2bc_sb[:].unsqueeze(1).broadcast_to([128, T, 128]),
                            OP.mult)
                        asr = sbs.tile([128, T], F32, tag="asr")
                        nc.vector.tensor_reduce(
                            asr[:], tmp[:].rearrange("p (t f) -> p t f", t=T),
                            mybir.AxisListType.X, OP.add)

                        ae = psm.tile([128, T * 2], F32, tag="ae")
                        adst_blk = adst2_all[:, b * 2:(b + 1) * 2]
                        for t in range(T):
                            nc.tensor.matmul(ae[:, t * 2:(t + 1) * 2],
                                             ST8_sb[:, t * 128:(t + 1) * 128],
                                             adst_blk, start=True, stop=True)

                        ae3 = ae[:].rearrange("p (t x) -> p t x", x=2)
                        ep1 = sbs.tile([128, T], F32, tag="ep1")
                        nc.vector.scalar_tensor_tensor(
                            ep1[:].unsqueeze(2), asr[:].unsqueeze(2), -meta["c2"],
                            ae3[:, :, 0:1], OP.add, OP.add)
                        ep2 = sbs.tile([128, T], F32, tag="ep2")
                        nc.vector.tensor_tensor(ep2[:].unsqueeze(2),
                                                ep1[:].unsqueeze(2),
                                                ae3[:, :, 1:2], OP.add)
                        lr = sbs.tile([128, T], F32, tag="lr")
                        nc.vector.scalar_tensor_tensor(lr[:], ep2[:], NEG, ep2[:],
                                                       OP.mult, OP.max)
                        p_all = sbs.tile([128, T], BF16, tag="p")
                        nc.scalar.activation(p_all[:], lr[:], AF.Exp)
                        # pair-duplicate p so the w multiply has a packed last dim
                        p_dup = sbs.tile([128, T * 2], BF16, tag="pdup")
                        nc.vector.tensor_copy(
                            p_dup[:].rearrange("p (t k) -> p t k", k=2),
                            p_all[:].unsqueeze(2).broadcast_to([128, T, 2]))

                        w_all = sbg.tile([128, T * 128], BF16, tag="w")
                        nc.vector.tensor_tensor(
                            w_all[:].rearrange("p (t j k) -> p t j k", t=T, k=2),
                            gat[:].rearrange("p t (j k) -> p t j k", k=2),
                            p_dup[:].rearrange("p (t k) -> p t k", t=T)
                                .unsqueeze(2).broadcast_to([128, T, 64, 2]),
                            OP.mult)

                        oacc = psb.tile([128, 128], F32, tag="oacc")
                        dacc = psb.tile([128, 1], F32, tag="dacc")
                        for t in range(T):
                            nc.tensor.matmul(oacc[:], S8_sb[:, t * 128:(t + 1) * 128],
                                             w_all[:, t * 128:(t + 1) * 128],
                                             start=(t == 0), stop=(t == T - 1))
                            nc.tensor.matmul(dacc[:], S8_sb[:, t * 128:(t + 1) * 128],
                                             p_all[:, t:t + 1],
                                             start=(t == 0), stop=(t == T - 1))

                        h2self = sbs.tile([128, 128], BF16, tag="h2self")
                        nc.sync.dma_start(h2self[:bs, :], hb2[base:base + bs, :])
                        selfw2 = sbs.tile([128, 128], F32, tag="selfw2")
                        nc.vector.tensor_single_scalar(
                            selfw2[:], h2self[:], pself2_all[:, b:b + 1], OP.mult)
                        osum2 = sbs.tile([128, 128], F32, tag="osum2")
                        nc.vector.tensor_tensor(osum2[:], oacc[:], selfw2[:], OP.add)
                        den = sbs.tile([128, 1], F32, tag="den")
                        nc.vector.scalar_tensor_tensor(
                            den[:], dacc[:], EPS, pself2_all[:, b:b + 1],
                            OP.add, OP.add)
                        rec = sbs.tile([128, 1], F32, tag="rec")
                        nc.vector.reciprocal(rec[:], den[:])
                        of = sbs.tile([128, 128], F32, tag="of")
                        nc.scalar.activation(of[:], osum2[:], AF.Copy,
                                             scale=rec[:, 0:1])
                        nc.sync.dma_start(out_d[base:base + bs, :], of[:bs, :])

    nc.compile()
    return nc


# --------------------------------------------------------------------------
# entry point
# --------------------------------------------------------------------------

def kernel(x, edge_index, W1, a_src1, a_dst1, b1, W2, a_src2, a_dst2, b2,
           _trace=False):
    in_maps, meta, perm_order = _prepare(
        x, edge_index, W1, a_src1, a_dst1, b1, W2, a_src2, a_dst2, b2)

    import time as _time
    _t0 = _time.time()
    key = (meta["TT"], tuple(meta["T_lo"]), tuple(meta["T_hi"]))
    if key not in _cache:
        _cache.clear()
        _cache[key] = _build(meta)
    nc = _cache[key]
    print(f"[kernel] build done at {_time.time()-_t0:.1f}s", flush=True)

    kw = {}
    if _trace:
        kw = dict(trace=True)
    res = bass_utils.run_bass_kernel_spmd(nc, in_maps, core_ids=list(range(C)), **kw)

    out = np.empty((N, HID), np.float32)
    for c in range(C):
        out[perm_order[c]] = res.results[c]["out"]
    kernel._last_result = res
    return out


# revision 13
# speedup vs baseline: 1.6380x; 1.0192x over previous
"""Trainium2 Bass kernel for nn_ClassDiagramGNN: 2-layer GAT on 50k nodes / 850k edges.

v4 design (8 NeuronCores, dst-sharded graph parallel, bf16 + fp8 one-hots):
  - Host: LPT-balance dst nodes into 128-blocks per core; physical node layout
    chunked [2, cores, rows] so each table AllGather splits into 2 overlapping
    collectives; non-loop edges bucketed by (core, block, src-chunk) padded to
    128-edge tiles; scatter one-hots S (edge->dst) / ST (dst->edge) in fp8.
  - Self-loop edges never gathered: their softmax terms are computed from
    local per-block data and added to numerator/denominator directly.
  - Layer-1 table rows: 1280B bf16 [h1+b1 (512, feat-major (f,h) interleave) |
    fp32 asrc (4) | pad]. Feat-major layout gives the p (x) h broadcast
    multiply a packed head-minor last dim (DVE 2x eligible).
  - Edge pass per dst block: dma_gather rows; per-tile ae = ST^T @ adst_hilo
    (fp8 x bf16 matmul); block-wide DVE logits (bitcast fp32 asrc view),
    leaky-relu, exp; one broadcast DVE multiply for w = p (x) h; aggregation
    oacc/dacc via S^T matmuls accumulated in PSUM; denominator post-applied.
  - Layer 2: 256B rows (h2@W2+b2); asrc2 recomputed on-chip (multiply +
    segmented reduce); pair-duplicated p2 keeps the multiply packed.
"""
import sys

for _p in ("/opt/trn_rl_repo",):
    if _p not in sys.path:
        sys.path.append(_p)

import heapq
import numpy as np
import ml_dtypes

import concourse.bass as bass
import concourse.bacc as bacc
import concourse.tile as tile
from concourse import mybir
from concourse import bass_utils

F32 = mybir.dt.float32
BF16 = mybir.dt.bfloat16
FP8 = mybir.dt.float8e4
I16 = mybir.dt.int16
AF = mybir.ActivationFunctionType
OP = mybir.AluOpType
BF16NP = ml_dtypes.bfloat16
FP8NP = ml_dtypes.float8_e4m3

# problem constants (hardcoded per contract)
N, F_IN, HID, H1, E = 50000, 512, 128, 4, 800000
NEG = 0.2
C = 8                 # cores
NS = N // C           # 6250 nodes per shard
NBLK = (NS + 127) // 128   # 49 blocks per core
CAPS = [128] * (NBLK - 1) + [NS - 128 * (NBLK - 1)]  # 48x128 + 106
CH0B = 24             # blocks in allgather chunk 0
CH0 = CH0B * 128      # 3072 rows/core in chunk 0
CH1 = NS - CH0        # 3178 rows/core in chunk 1
HALF = C * CH0        # 24576: phys row where chunk 1 starts (gather halves)
ROW1 = 640            # layer-1 row: 512 bf16 h (feat-major) | 8 slots fp32 asrc | pad
ROW2 = 128            # layer-2 row: 128 bf16 h (256B)
EPS = 1e-16

_cache = {}


def _reconfigure(n, e):
    """Testing hook: shrink the graph (keeps F_IN/HID/H1 fixed)."""
    global N, E, NS, NBLK, CAPS, CH0B, CH0, CH1, HALF
    N, E = n, e
    NS = N // C
    NBLK = (NS + 127) // 128
    CAPS = [128] * (NBLK - 1) + [NS - 128 * (NBLK - 1)]
    CH0B = NBLK // 2
    CH0 = CH0B * 128
    CH1 = NS - CH0
    HALF = C * CH0
    _cache.clear()


# --------------------------------------------------------------------------
# host-side preprocessing
# --------------------------------------------------------------------------

def _prepare(x, edge_index, W1, a_src1, a_dst1, b1, W2, a_src2, a_dst2, b2):
    # self-loop edges are handled by a local (gather-free) path on device
    src = edge_index[0].astype(np.int64)
    dst = edge_index[1].astype(np.int64)
    deg = np.bincount(dst, minlength=N) + 1

    # nodes -> cores: snake order by degree (equalizes per-core edge load)
    sort_by_deg = np.argsort(-deg, kind="stable")
    core_of = np.empty(N, dtype=np.int64)
    snake = np.tile(np.concatenate([np.arange(C), np.arange(C)[::-1]]),
                    (N // (2 * C)) + 1)[:N]
    core_of[sort_by_deg] = snake

    # Approximate per-node lo/hi degree split is unknowable before the core
    # assignment is final (lo/hi depends on src physical rows, which depend on
    # everyone's placement). Iterate once: place by total degree, compute the
    # split, re-place blocks with (lo,hi)-aware LPT.
    local_r = np.empty(N, dtype=np.int64)
    perm_order = np.empty((C, NS), dtype=np.int64)

    def place(lo_d, hi_d):
        for c in range(C):
            ids = np.where(core_of == c)[0]
            tot = lo_d[ids] + hi_d[ids]
            order = np.argsort(-tot, kind="stable")
            loads = np.zeros((NBLK, 2))
            used = np.zeros(NBLK, np.int64)
            assign = [[] for _ in range(NBLK)]
            for j in order:
                oid = ids[j]
                best, bestv = -1, None
                for bi in range(NBLK):
                    if used[bi] >= CAPS[bi]:
                        continue
                    v = max(loads[bi, 0] + lo_d[oid], loads[bi, 1] + hi_d[oid])
                    if bestv is None or v < bestv:
                        best, bestv = bi, v
                assign[best].append(oid)
                loads[best, 0] += lo_d[oid]
                loads[best, 1] += hi_d[oid]
                used[best] += 1
            pos = 0
            for bi in range(NBLK):
                for oid in assign[bi]:
                    local_r[oid] = pos
                    perm_order[c, pos] = oid
                    pos += 1

    # pass 1: split unknown -> put everything in lo
    place(deg.astype(np.float64), np.zeros(N))
    r = local_r
    phys = np.where(r < CH0, core_of * CH0 + r,
                    HALF + core_of * CH1 + (r - CH0))
    # actual lo/hi degree per dst node under this placement
    hi_e = (phys[src] >= HALF).astype(np.int64)
    lo_deg = np.bincount(dst, weights=1 - hi_e, minlength=N)
    hi_deg = np.bincount(dst, weights=hi_e, minlength=N)
    # self-loop counts toward neither (local path)
    place(lo_deg, hi_deg)

    # physical (chunked) row layout for the gather tables
    r = local_r
    phys = np.where(r < CH0, core_of * CH0 + r,
                    HALF + core_of * CH1 + (r - CH0))

    src_p = phys[src]
    dcore = core_of[dst]
    dloc = local_r[dst]
    blk = dloc // 128
    dcol = dloc % 128
    halfv = (src_p >= HALF).astype(np.int64)
    key = (dcore * NBLK + blk) * 2 + halfv
    eorder = np.argsort(key, kind="stable")
    counts = np.bincount(key, minlength=C * NBLK * 2).reshape(C, NBLK, 2)

    T_lo = -(-counts[:, :, 0].max(axis=0) // 128)  # ceil, uniform across cores
    T_hi = -(-counts[:, :, 1].max(axis=0) // 128)
    T_all = T_lo + T_hi
    TT = int(T_all.sum())
    toff = np.zeros(NBLK, np.int64)
    toff[1:] = np.cumsum(T_all)[:-1]

    src_sorted = src_p[eorder]
    dcol_sorted = dcol[eorder]
    starts = np.zeros(C * NBLK * 2 + 1, np.int64)
    starts[1:] = np.cumsum(counts.reshape(-1))

    idx_all = np.zeros((C, TT * 128), np.int16)           # pad -> row 0
    dc_all = np.full((C, TT * 128), -1, np.int64)         # pad -> -1
    for c in range(C):
        for b in range(NBLK):
            for h in range(2):
                k = (c * NBLK + b) * 2 + h
                s0, s1 = starts[k], starts[k + 1]
                n = s1 - s0
                if n == 0:
                    continue
                slot0 = (toff[b] + (T_lo[b] if h else 0)) * 128
                seg = src_sorted[s0:s1]
                if h:
                    seg = seg - HALF
                idx_all[c, slot0:slot0 + n] = seg.astype(np.int16)
                dc_all[c, slot0:slot0 + n] = dcol_sorted[s0:s1]

    # one-hot scatter matrices in fp8 (exact 0.0 / 1.0)
    slots = np.arange(TT * 128)
    t_of = slots // 128
    ep_of = slots % 128
    ONE8 = np.float32(1.0).astype(FP8NP).view(np.uint8)  # 0x38

    # weights: fold attention projections in; feat-major (f,h) column order
    W1_64 = np.asarray(W1, np.float64)
    Dsrc1 = np.zeros((H1 * HID, H1), np.float64)
    Ddst1 = np.zeros((H1 * HID, H1), np.float64)
    for h in range(H1):
        Dsrc1[h * HID:(h + 1) * HID, h] = np.asarray(a_src1, np.float64)[h]
        Ddst1[h * HID:(h + 1) * HID, h] = np.asarray(a_dst1, np.float64)[h]
    # feat-major index: new position f*4+h  <- old position h*128+f
    ff = np.arange(512)
    fmaj = (ff % 4) * 128 + ff // 4   # old index stored at new slot ff
    rhs1 = np.concatenate(
        [W1_64[:, fmaj], W1_64 @ Dsrc1, W1_64 @ Ddst1], axis=1).astype(BF16NP)
    W2_64 = np.asarray(W2, np.float64)  # rows indexed by old h*128+f order
    rhs2 = np.concatenate(
        [W2_64[fmaj, :],
         (W2_64 @ np.asarray(a_dst2, np.float64)[0][:, None])[fmaj, :]],
        axis=1).astype(BF16NP)                            # [512 (f-major), 129]

    b1_fm = np.asarray(b1, np.float64)[fmaj]
    b1_bc = np.tile(b1_fm[None, :], (128, 1)).astype(BF16NP)
    b2_bc = np.tile(np.asarray(b2, np.float32)[None, :], (128, 1)).astype(BF16NP)
    asrc2_bc = np.tile(np.asarray(a_src2, np.float32)[0][None, :],
                       (128, 1)).astype(BF16NP)
    c2 = float(np.asarray(b2, np.float64) @ np.asarray(a_src2, np.float64)[0])
    ident = np.eye(128, dtype=np.float32)

    xnp = np.asarray(x, np.float32)
    in_maps = []
    for c in range(C):
        xT = np.ascontiguousarray(xnp[perm_order[c]].T).astype(BF16NP)  # [512, NS]
        idx_w = np.ascontiguousarray(
            np.tile(idx_all[c].reshape(-1, 16).T, (8, 1)))  # [128, TT*8]
        dc = dc_all[c]
        valid = dc >= 0
        S8 = np.zeros((128, TT * 128), np.uint8)
        S8[ep_of[valid], t_of[valid] * 128 + dc[valid]] = ONE8
        ST8 = np.zeros((128, TT * 128), np.uint8)
        ST8[dc[valid], t_of[valid] * 128 + ep_of[valid]] = ONE8
        in_maps.append({
            "xT": xT, "rhs1": rhs1, "rhs2": rhs2,
            "b1bc": b1_bc, "b2bc": b2_bc, "asrc2bc": asrc2_bc,
            "identf": ident,
            "idx": idx_w,
            "S8": S8.view(FP8NP), "ST8": ST8.view(FP8NP),
        })

    meta = {
        "T_lo": [int(v) for v in T_lo],
        "T_hi": [int(v) for v in T_hi],
        "toff": [int(v) for v in toff],
        "TT": TT,
        "c2": c2,
    }
    return in_maps, meta, perm_order


# --------------------------------------------------------------------------
# device program
# --------------------------------------------------------------------------

def _build(meta):
    nc = bacc.Bacc("TRN2", target_bir_lowering=False, debug=False, num_devices=C)
    TT = meta["TT"]

    xT_d = nc.dram_tensor("xT", [F_IN, NS], BF16, kind="ExternalInput").ap()
    rhs1_d = nc.dram_tensor("rhs1", [F_IN, 520], BF16, kind="ExternalInput").ap()
    rhs2_d = nc.dram_tensor("rhs2", [F_IN, 129], BF16, kind="ExternalInput").ap()
    b1bc_d = nc.dram_tensor("b1bc", [128, 512], BF16, kind="ExternalInput").ap()
    b2bc_d = nc.dram_tensor("b2bc", [128, 128], BF16, kind="ExternalInput").ap()
    asrc2bc_d = nc.dram_tensor("asrc2bc", [128, 128], BF16, kind="ExternalInput").ap()
    identf_d = nc.dram_tensor("identf", [128, 128], F32, kind="ExternalInput").ap()
    idx_d = nc.dram_tensor("idx", [128, TT * 8], I16, kind="ExternalInput").ap()
    S8_d = nc.dram_tensor("S8", [128, TT * 128], FP8, kind="ExternalInput").ap()
    ST8_d = nc.dram_tensor("ST8", [128, TT * 128], FP8, kind="ExternalInput").ap()
    out_d = nc.dram_tensor("out", [NS, HID], F32, kind="ExternalOutput").ap()

    groups = [list(range(C))]

    with tile.TileContext(nc, num_cores=C) as tc:
        with tc.tile_pool(name="dram", bufs=1, space="DRAM") as dram:
            hb1 = dram.tile([NS, ROW1], BF16)
            hfull1a = dram.tile([HALF, ROW1], BF16, addr_space="Shared")
            hfull1b = dram.tile([N - HALF, ROW1], BF16, addr_space="Shared")
            hb2 = dram.tile([NS, ROW2], BF16)
            hfull2a = dram.tile([HALF, ROW2], BF16, addr_space="Shared")
            hfull2b = dram.tile([N - HALF, ROW2], BF16, addr_space="Shared")

            with tc.tile_pool(name="lv", bufs=1) as lv:
                rhs1_sb = []
                rhs2_sb = []
                for k in range(4):
                    rt = lv.tile([128, 520], BF16, name=f"rhs1sb{k}")
                    nc.sync.dma_start(rt[:], rhs1_d[k * 128:(k + 1) * 128, :])
                    rhs1_sb.append(rt)
                    rt2 = lv.tile([128, 129], BF16, name=f"rhs2sb{k}")
                    nc.sync.dma_start(rt2[:], rhs2_d[k * 128:(k + 1) * 128, :])
                    rhs2_sb.append(rt2)
                b1bc_sb = lv.tile([128, 512], BF16, name="b1bc")
                nc.sync.dma_start(b1bc_sb[:], b1bc_d)
                b2bc_sb = lv.tile([128, 128], BF16, name="b2bc")
                nc.sync.dma_start(b2bc_sb[:], b2bc_d)
                asrc2bc_sb = lv.tile([128, 128], BF16, name="asrc2bc")
                nc.sync.dma_start(asrc2bc_sb[:], asrc2bc_d)
                identf_sb = lv.tile([128, 128], F32, name="identf")
                nc.sync.dma_start(identf_sb[:], identf_d)
                adst1_all = lv.tile([128, NBLK * 8], BF16, name="adst1")
                adst2_all = lv.tile([128, NBLK * 2], BF16, name="adst2")
                pself1_all = lv.tile([128, NBLK * 4], F32, name="pself1")
                pself2_all = lv.tile([128, NBLK], F32, name="pself2")

                # ---------------- phase A: h1 shard + attn scalars ------------
                with (
                    tc.tile_pool(name="a_w", bufs=4) as sbw,
                    tc.tile_pool(name="a_p", bufs=3, space="PSUM") as psp,
                ):
                    for b in range(NBLK):
                        bs = CAPS[b]
                        base = b * 128
                        ph = psp.tile([128, 512], F32, tag="ph")
                        pa = psp.tile([128, 8], F32, tag="pa")
                        xt = sbw.tile([128, 4, 128], BF16, tag="xt")
                        nc.sync.dma_start(
                            xt[:, :, :bs],
                            xT_d[:, base:base + bs].rearrange(
                                "(k p) c -> p k c", k=4))
                        for k in range(4):
                            nc.tensor.matmul(ph[:bs, :], xt[:, k, :bs],
                                             rhs1_sb[k][:, 0:512],
                                             start=(k == 0), stop=(k == 3))
                            nc.tensor.matmul(pa[:bs, :], xt[:, k, :bs],
                                             rhs1_sb[k][:, 512:520],
                                             start=(k == 0), stop=(k == 3))
                        ha = sbw.tile([128, 512], BF16, tag="ha")
                        nc.vector.tensor_tensor(ha[:bs, :], ph[:bs, :],
                                                b1bc_sb[:bs, :], OP.add)
                        asx = sbw.tile([128, 4], F32, tag="asx")
                        nc.scalar.activation(asx[:bs], pa[:bs, 0:4], AF.Copy)
                        adh = adst1_all[:, b * 8:b * 8 + 4]
                        adl = adst1_all[:, b * 8 + 4:b * 8 + 8]
                        if bs < 128:
                            nc.vector.memset(adst1_all[:, b * 8:(b + 1) * 8], 0.0)
                        nc.scalar.activation(adh[:bs], pa[:bs, 4:8], AF.Copy)
                        nc.vector.tensor_tensor(adl[:bs], pa[:bs, 4:8], adh[:bs],
                                                OP.subtract)
                        # self-loop logits: p = exp(leaky(asrc_i + adst_i))
                        eps_ = sbw.tile([128, 4], F32, tag="eps_")
                        nc.vector.tensor_tensor(eps_[:bs], asx[:bs],
                                                pa[:bs, 4:8], OP.add)
                        lrs = sbw.tile([128, 4], F32, tag="lrs")
                        nc.vector.scalar_tensor_tensor(lrs[:bs], eps_[:bs], NEG,
                                                       eps_[:bs], OP.mult, OP.max)
                        if bs < 128:
                            nc.vector.memset(pself1_all[:, b * 4:(b + 1) * 4], 0.0)
                        nc.scalar.activation(pself1_all[:bs, b * 4:(b + 1) * 4],
                                             lrs[:bs], AF.Exp)
                        nc.sync.dma_start(hb1[base:base + bs, 0:512], ha[:bs, :])
                        nc.sync.dma_start(hb1[base:base + bs, 512:520],
                                          asx[:bs, :].bitcast(BF16))
                        if b == CH0B - 1:
                            nc.gpsimd.collective_compute(
                                "AllGather", OP.bypass, replica_groups=groups,
                                ins=[hb1[0:CH0, :].opt()],
                                outs=[hfull1a[:].opt()])
                nc.gpsimd.collective_compute(
                    "AllGather", OP.bypass, replica_groups=groups,
                    ins=[hb1[CH0:NS, :].opt()], outs=[hfull1b[:].opt()])

                # ---------------- phase B: layer-1 edge pass + h2@W2 ----------
                with (
                    tc.tile_pool(name="b_m", bufs=3) as sbm,
                    tc.tile_pool(name="b_g", bufs=2) as sbg,
                    tc.tile_pool(name="b_s", bufs=2) as sbs,
                    tc.tile_pool(name="b_w", bufs=2) as sbw2,
                    tc.tile_pool(name="b_pb", bufs=2, space="PSUM") as psb,
                    tc.tile_pool(name="b_pm", bufs=2, space="PSUM") as psm,
                    tc.tile_pool(name="b_ph", bufs=1, space="PSUM") as psh,
                ):
                    for b in range(NBLK):
                        bs = CAPS[b]
                        base = b * 128
                        T_lo, T_hi = meta["T_lo"][b], meta["T_hi"][b]
                        T = T_lo + T_hi
                        boff = meta["toff"][b]

                        idx_sb = sbm.tile([128, T * 8], I16, tag="idx")
                        nc.sync.dma_start(idx_sb[:], idx_d[:, boff * 8:(boff + T) * 8])
                        S8_sb = sbm.tile([128, T * 128], FP8, tag="S8")
                        nc.sync.dma_start(S8_sb[:], S8_d[:, boff * 128:(boff + T) * 128])
                        ST8_sb = sbm.tile([128, T * 128], FP8, tag="ST8")
                        nc.sync.dma_start(ST8_sb[:], ST8_d[:, boff * 128:(boff + T) * 128])

                        gat = sbg.tile([128, T, ROW1], BF16, tag="gat")
                        if T_lo:
                            nc.gpsimd.dma_gather(
                                gat[:, 0:T_lo, :], hfull1a[:],
                                idx_sb[:, 0:T_lo * 8], T_lo * 128, T_lo * 128,
                                ROW1, elem_step=ROW1, single_packet=False)
                        if T_hi:
                            nc.gpsimd.dma_gather(
                                gat[:, T_lo:T, :], hfull1b[:],
                                idx_sb[:, T_lo * 8:T * 8], T_hi * 128, T_hi * 128,
                                ROW1, elem_step=ROW1, single_packet=False)

                        ae = psm.tile([128, T * 8], F32, tag="ae")
                        adst_blk = adst1_all[:, b * 8:(b + 1) * 8]
                        for t in range(T):
                            nc.tensor.matmul(ae[:, t * 8:(t + 1) * 8],
                                             ST8_sb[:, t * 128:(t + 1) * 128],
                                             adst_blk, start=True, stop=True)

                        asrc_v = gat[:, :, 512:520].bitcast(F32)      # [128,T,4]
                        ae3 = ae[:].rearrange("p (t x) -> p t x", x=8)
                        ep1 = sbs.tile([128, T * 4], F32, tag="ep1")
                        nc.vector.tensor_tensor(
                            ep1[:].rearrange("p (t h) -> p t h", h=4),
                            asrc_v, ae3[:, :, 0:4], OP.add)
                        ep2 = sbs.tile([128, T * 4], F32, tag="ep2")
                        nc.vector.tensor_tensor(
                            ep2[:].rearrange("p (t h) -> p t h", h=4),
                            ep1[:].rearrange("p (t h) -> p t h", h=4),
                            ae3[:, :, 4:8], OP.add)
                        lr = sbs.tile([128, T * 4], F32, tag="lr")
                        nc.vector.scalar_tensor_tensor(lr[:], ep2[:], NEG, ep2[:],
                                                       OP.mult, OP.max)
                        p_all = sbs.tile([128, T * 4], BF16, tag="p")
                        nc.scalar.activation(p_all[:], lr[:], AF.Exp)

                        # w[e, t, f, h] = h[e, t, f, h] * p[e, t, h] (feat-major)
                        w_all = sbg.tile([128, T * 512], BF16, tag="w")
                        nc.vector.tensor_tensor(
                            w_all[:].rearrange("p (t f h) -> p t f h", t=T, h=4),
                            gat[:, :, 0:512].rearrange("p t (f h) -> p t f h", h=4),
                            p_all[:].rearrange("p (t h) -> p t h", t=T)
                                .unsqueeze(2).broadcast_to([128, T, 128, 4]),
                            OP.mult)

                        oacc = psb.tile([128, 512], F32, tag="oacc")
                        dacc = psb.tile([128, 4], F32, tag="dacc")
                        for t in range(T):
                            nc.tensor.matmul(oacc[:], S8_sb[:, t * 128:(t + 1) * 128],
                                             w_all[:, t * 512:(t + 1) * 512],
                                             start=(t == 0), stop=(t == T - 1))
                            nc.tensor.matmul(dacc[:], S8_sb[:, t * 128:(t + 1) * 128],
                                             p_all[:, t * 4:(t + 1) * 4],
                                             start=(t == 0), stop=(t == T - 1))

                        # block writer: self-loop add, normalize, ELU, h2 @ rhs2
                        hself = sbw2.tile([128, 512], BF16, tag="hself")
                        nc.sync.dma_start(hself[:bs, :], hb1[base:base + bs, 0:512])
                        selfw = sbw2.tile([128, 512], F32, tag="selfw")
                        nc.vector.tensor_tensor(
                            selfw[:].rearrange("p (f h) -> p f h", h=4),
                            hself[:].rearrange("p (f h) -> p f h", h=4),
                            pself1_all[:, b * 4:(b + 1) * 4]
                                .unsqueeze(1).broadcast_to([128, 128, 4]),
                            OP.mult)
                        osum = sbw2.tile([128, 512], F32, tag="osum")
                        nc.vector.tensor_tensor(osum[:], oacc[:], selfw[:], OP.add)
                        den = sbs.tile([128, 4], F32, tag="den")
                        nc.vector.scalar_tensor_tensor(
                            den[:], dacc[:], EPS,
                            pself1_all[:, b * 4:(b + 1) * 4], OP.add, OP.add)
                        rec = sbs.tile([128, 4], F32, tag="rec")
                        nc.vector.reciprocal(rec[:], den[:])
                        h2 = sbw2.tile([128, 512], F32, tag="h2")
                        nc.vector.tensor_tensor(
                            h2[:].rearrange("p (f h) -> p f h", h=4),
                            osum[:].rearrange("p (f h) -> p f h", h=4),
                            rec[:].unsqueeze(1).broadcast_to([128, 128, 4]),
                            OP.mult)
                        rl = sbw2.tile([128, 512], F32, tag="rl")
                        nc.scalar.activation(rl[:], h2[:], AF.Relu)
                        mn = sbw2.tile([128, 512], F32, tag="mn")
                        nc.vector.tensor_scalar_min(mn[:], h2[:], 0.0)
                        em = sbw2.tile([128, 512], F32, tag="em")
                        nc.scalar.activation(em[:], mn[:], AF.Exp)
                        h2f = sbw2.tile([128, 512], F32, tag="h2f")
                        nc.vector.scalar_tensor_tensor(h2f[:], em[:], -1.0, rl[:],
                                                       OP.add, OP.add)
                        hh = psh.tile([128, 129], F32, tag="hh")
                        for k in range(4):
                            tp = psm.tile([128, 128], F32, tag="tp", bufs=1)
                            nc.tensor.transpose(tp[:], h2f[:, k * 128:(k + 1) * 128],
                                                identf_sb[:])
                            h2T = sbs.tile([128, 128], BF16, tag="h2T")
                            nc.scalar.activation(h2T[:], tp[:], AF.Copy)
                            nc.tensor.matmul(hh[:], h2T[:], rhs2_sb[k][:],
                                             start=(k == 0), stop=(k == 3))
                        ha2 = sbw2.tile([128, 128], BF16, tag="ha2")
                        nc.vector.tensor_tensor(ha2[:bs, :], hh[:bs, 0:128],
                                                b2bc_sb[:bs, :], OP.add)
                        ad2h = adst2_all[:, b * 2:b * 2 + 1]
                        ad2l = adst2_all[:, b * 2 + 1:b * 2 + 2]
                        if bs < 128:
                            nc.vector.memset(adst2_all[:, b * 2:(b + 1) * 2], 0.0)
                        nc.scalar.activation(ad2h[:bs], hh[:bs, 128:129], AF.Copy)
                        nc.vector.tensor_tensor(ad2l[:bs], hh[:bs, 128:129],
                                                ad2h[:bs], OP.subtract)
                        # self-loop p2 = exp(leaky(asrc2_i + adst2_i))
                        tt2 = sbw2.tile([128, 128], BF16, tag="tt2")
                        nc.vector.tensor_tensor(tt2[:], ha2[:], asrc2bc_sb[:],
                                                OP.mult)
                        as2 = sbw2.tile([128, 1], F32, tag="as2")
                        nc.vector.tensor_reduce(as2[:], tt2[:],
                                                mybir.AxisListType.X, OP.add)
                        ep2s = sbw2.tile([128, 1], F32, tag="ep2s")
                        nc.vector.scalar_tensor_tensor(ep2s[:bs], as2[:bs],
                                                       -meta["c2"],
                                                       hh[:bs, 128:129],
                                                       OP.add, OP.add)
                        lr2s = sbw2.tile([128, 1], F32, tag="lr2s")
                        nc.vector.scalar_tensor_tensor(lr2s[:bs], ep2s[:bs], NEG,
                                                       ep2s[:bs], OP.mult, OP.max)
                        if bs < 128:
                            nc.vector.memset(pself2_all[:, b:b + 1], 0.0)
                        nc.scalar.activation(pself2_all[:bs, b:b + 1], lr2s[:bs],
                                             AF.Exp)
                        nc.sync.dma_start(hb2[base:base + bs, :], ha2[:bs, :])
                        if b == CH0B - 1:
                            nc.gpsimd.collective_compute(
                                "AllGather", OP.bypass, replica_groups=groups,
                                ins=[hb2[0:CH0, :].opt()],
                                outs=[hfull2a[:].opt()])
                nc.gpsimd.collective_compute(
                    "AllGather", OP.bypass, replica_groups=groups,
                    ins=[hb2[CH0:NS, :].opt()], outs=[hfull2b[:].opt()])

                # ---------------- phase D: layer-2 edge pass ------------------
                with (
                    tc.tile_pool(name="d_m", bufs=3) as sbm,
                    tc.tile_pool(name="d_g", bufs=3) as sbg,
                    tc.tile_pool(name="d_s", bufs=3) as sbs,
                    tc.tile_pool(name="d_pb", bufs=2, space="PSUM") as psb,
                    tc.tile_pool(name="d_pm", bufs=2, space="PSUM") as psm,
                ):
                    for b in range(NBLK):
                        bs = CAPS[b]
                        base = b * 128
                        T_lo, T_hi = meta["T_lo"][b], meta["T_hi"][b]
                        T = T_lo + T_hi
                        boff = meta["toff"][b]

                        idx_sb = sbm.tile([128, T * 8], I16, tag="idx")
                        nc.sync.dma_start(idx_sb[:], idx_d[:, boff * 8:(boff + T) * 8])
                        S8_sb = sbm.tile([128, T * 128], FP8, tag="S8")
                        nc.sync.dma_start(S8_sb[:], S8_d[:, boff * 128:(boff + T) * 128])
                        ST8_sb = sbm.tile([128, T * 128], FP8, tag="ST8")
                        nc.sync.dma_start(ST8_sb[:], ST8_d[:, boff * 128:(boff + T) * 128])

                        gat = sbg.tile([128, T, ROW2], BF16, tag="gat")
                        if T_lo:
                            nc.gpsimd.dma_gather(
                                gat[:, 0:T_lo, :], hfull2a[:],
                                idx_sb[:, 0:T_lo * 8], T_lo * 128, T_lo * 128,
                                ROW2, elem_step=ROW2, single_packet=False)
                        if T_hi:
                            nc.gpsimd.dma_gather(
                                gat[:, T_lo:T, :], hfull2b[:],
                                idx_sb[:, T_lo * 8:T * 8], T_hi * 128, T_hi * 128,
                                ROW2, elem_step=ROW2, single_packet=False)

                        tmp = sbg.tile([128, T * 128], BF16, tag="tmp")
                        nc.vector.tensor_tensor(
                            tmp[:].rearrange("p (t f) -> p t f", t=T),
                            gat[:],
                            asrc2bc_sb[:].unsqueeze(1).broadcast_to([128, T, 128]),
                            OP.mult)
                        asr = sbs.tile([128, T], F32, tag="asr")
                        nc.vector.tensor_reduce(
                            asr[:], tmp[:].rearrange("p (t f) -> p t f", t=T),
                            mybir.AxisListType.X, OP.add)

                        ae = psm.tile([128, T * 2], F32, tag="ae")
                        adst_blk = adst2_all[:, b * 2:(b + 1) * 2]
                        for t in range(T):
                            nc.tensor.matmul(ae[:, t * 2:(t + 1) * 2],
                                             ST8_sb[:, t * 128:(t + 1) * 128],
                                             adst_blk, start=True, stop=True)

                        ae3 = ae[:].rearrange("p (t x) -> p t x", x=2)
                        ep1 = sbs.tile([128, T], F32, tag="ep1")
                        nc.vector.scalar_tensor_tensor(
                            ep1[:].unsqueeze(2), asr[:].unsqueeze(2), -meta["c2"],
                            ae3[:, :, 0:1], OP.add, OP.add)
                        ep2 = sbs.tile([128, T], F32, tag="ep2")
                        nc.vector.tensor_tensor(ep2[:].unsqueeze(2),
                                                ep1[:].unsqueeze(2),
                                                ae3[:, :, 1:2], OP.add)
                        lr = sbs.tile([128, T], F32, tag="lr")
                        nc.vector.scalar_tensor_tensor(lr[:], ep2[:], NEG, ep2[:],
                                                       OP.mult, OP.max)
                        p_all = sbs.tile([128, T], BF16, tag="p")
                        nc.scalar.activation(p_all[:], lr[:], AF.Exp)
                        # pair-duplicate p so the w multiply has a packed last dim
                        p_dup = sbs.tile([128, T * 2], BF16, tag="pdup")
                        nc.vector.tensor_copy(
                            p_dup[:].rearrange("p (t k) -> p t k", k=2),
                            p_all[:].unsqueeze(2).broadcast_to([128, T, 2]))

                        w_all = sbg.tile([128, T * 128], BF16, tag="w")
                        nc.vector.tensor_tensor(
                            w_all[:].rearrange("p (t j k) -> p t j k", t=T, k=2),
                            gat[:].rearrange("p t (j k) -> p t j k", k=2),
                            p_dup[:].rearrange("p (t k) -> p t k", t=T)
                                .unsqueeze(2).broadcast_to([128, T, 64, 2]),
                            OP.mult)

                        oacc = psb.tile([128, 128], F32, tag="oacc")
                        dacc = psb.tile([128, 1], F32, tag="dacc")
                        for t in range(T):
                            nc.tensor.matmul(oacc[:], S8_sb[:, t * 128:(t + 1) * 128],
                                             w_all[:, t * 128:(t + 1) * 128],
                                             start=(t == 0), stop=(t == T - 1))
                            nc.tensor.matmul(dacc[:], S8_sb[:, t * 128:(t + 1) * 128],
                                             p_all[:, t:t + 1],
                                             start=(t == 0), stop=(t == T - 1))

                        h2self = sbs.tile([128, 128], BF16, tag="h2self")
                        nc.sync.dma_start(h2self[:bs, :], hb2[base:base + bs, :])
                        selfw2 = sbs.tile([128, 128], F32, tag="selfw2")
                        nc.vector.tensor_single_scalar(
                            selfw2[:], h2self[:], pself2_all[:, b:b + 1], OP.mult)
                        osum2 = sbs.tile([128, 128], F32, tag="osum2")
                        nc.vector.tensor_tensor(osum2[:], oacc[:], selfw2[:], OP.add)
                        den = sbs.tile([128, 1], F32, tag="den")
                        nc.vector.scalar_tensor_tensor(
                            den[:], dacc[:], EPS, pself2_all[:, b:b + 1],
                            OP.add, OP.add)
                        rec = sbs.tile([128, 1], F32, tag="rec")
                        nc.vector.reciprocal(rec[:], den[:])
                        of = sbs.tile([128, 128], F32, tag="of")
                        nc.scalar.activation(of[:], osum2[:], AF.Copy,
                                             scale=rec[:, 0:1])
                        nc.sync.dma_start(out_d[base:base + bs, :], of[:bs, :])

    nc.compile()
    return nc


# --------------------------------------------------------------------------
# entry point
# --------------------------------------------------------------------------

def kernel(x, edge_index, W1, a_src1, a_dst1, b1, W2, a_src2, a_dst2, b2,
           _trace=False):
    in_maps, meta, perm_order = _prepare(
        x, edge_index, W1, a_src1, a_dst1, b1, W2, a_src2, a_dst2, b2)

    import time as _time
    _t0 = _time.time()
    key = (meta["TT"], tuple(meta["T_lo"]), tuple(meta["T_hi"]))
    if key not in _cache:
        _cache.clear()
        _cache[key] = _build(meta)
    nc = _cache[key]
    print(f"[kernel] build done at {_time.time()-_t0:.1f}s", flush=True)

    kw = {}
    if _trace:
        kw = dict(trace=True)
    res = bass_utils.run_bass_kernel_spmd(nc, in_maps, core_ids=list(range(C)), **kw)

    out = np.empty((N, HID), np.float32)
    for c in range(C):
        out[perm_order[c]] = res.results[c]["out"]
    kernel._last_result = res
    return out


# revision 17
# speedup vs baseline: 1.7856x; 1.0901x over previous
"""Trainium2 Bass kernel for nn_ClassDiagramGNN: 2-layer GAT on 50k nodes / 850k edges.

v4 design (8 NeuronCores, dst-sharded graph parallel, bf16 + fp8 one-hots):
  - Host: LPT-balance dst nodes into 128-blocks per core; physical node layout
    chunked [2, cores, rows] so each table AllGather splits into 2 overlapping
    collectives; non-loop edges bucketed by (core, block, src-chunk) padded to
    128-edge tiles; scatter one-hots S (edge->dst) / ST (dst->edge) in fp8.
  - Self-loop edges never gathered: their softmax terms are computed from
    local per-block data and added to numerator/denominator directly.
  - Layer-1 table rows: 1280B bf16 [h1+b1 (512, feat-major (f,h) interleave) |
    fp32 asrc (4) | pad]. Feat-major layout gives the p (x) h broadcast
    multiply a packed head-minor last dim (DVE 2x eligible).
  - Edge pass per dst block: dma_gather rows; per-tile ae = ST^T @ adst_hilo
    (fp8 x bf16 matmul); block-wide DVE logits (bitcast fp32 asrc view),
    leaky-relu, exp; one broadcast DVE multiply for w = p (x) h; aggregation
    oacc/dacc via S^T matmuls accumulated in PSUM; denominator post-applied.
  - Layer 2: 256B rows (h2@W2+b2); asrc2 recomputed on-chip (multiply +
    segmented reduce); pair-duplicated p2 keeps the multiply packed.
"""
import sys

for _p in ("/opt/trn_rl_repo",):
    if _p not in sys.path:
        sys.path.append(_p)

import heapq
import numpy as np
import ml_dtypes

import concourse.bass as bass
import concourse.bacc as bacc
import concourse.tile as tile
from concourse import mybir
from concourse import bass_utils

F32 = mybir.dt.float32
BF16 = mybir.dt.bfloat16
FP8 = mybir.dt.float8e4
I16 = mybir.dt.int16
AF = mybir.ActivationFunctionType
OP = mybir.AluOpType
BF16NP = ml_dtypes.bfloat16
FP8NP = ml_dtypes.float8_e4m3

# problem constants (hardcoded per contract)
N, F_IN, HID, H1, E = 50000, 512, 128, 4, 800000
NEG = 0.2
C = 8                 # cores
NS = N // C           # 6250 nodes per shard
NBLK = (NS + 127) // 128   # 49 blocks per core
CAPS = [128] * (NBLK - 1) + [NS - 128 * (NBLK - 1)]  # 48x128 + 106
CH0B = 24             # blocks in allgather chunk 0
CH0 = CH0B * 128      # 3072 rows/core in chunk 0
CH1 = NS - CH0        # 3178 rows/core in chunk 1
HALF = C * CH0        # 24576: phys row where chunk 1 starts (gather halves)
ROW1 = 640            # layer-1 row: 512 bf16 h (feat-major) | 8 slots fp32 asrc | pad
ROW2 = 128            # layer-2 row: 128 bf16 h (256B)
EPS = 1e-16

_cache = {}


def _reconfigure(n, e):
    """Testing hook: shrink the graph (keeps F_IN/HID/H1 fixed)."""
    global N, E, NS, NBLK, CAPS, CH0B, CH0, CH1, HALF
    N, E = n, e
    NS = N // C
    NBLK = (NS + 127) // 128
    CAPS = [128] * (NBLK - 1) + [NS - 128 * (NBLK - 1)]
    CH0B = NBLK // 2
    CH0 = CH0B * 128
    CH1 = NS - CH0
    HALF = C * CH0
    _cache.clear()


# --------------------------------------------------------------------------
# host-side preprocessing
# --------------------------------------------------------------------------

def _prepare(x, edge_index, W1, a_src1, a_dst1, b1, W2, a_src2, a_dst2, b2):
    # self-loop edges are handled by a local (gather-free) path on device
    src = edge_index[0].astype(np.int64)
    dst = edge_index[1].astype(np.int64)
    deg = np.bincount(dst, minlength=N) + 1

    # nodes -> cores: snake order by degree (equalizes per-core edge load)
    sort_by_deg = np.argsort(-deg, kind="stable")
    core_of = np.empty(N, dtype=np.int64)
    snake = np.tile(np.concatenate([np.arange(C), np.arange(C)[::-1]]),
                    (N // (2 * C)) + 1)[:N]
    core_of[sort_by_deg] = snake

    # Approximate per-node lo/hi degree split is unknowable before the core
    # assignment is final (lo/hi depends on src physical rows, which depend on
    # everyone's placement). Iterate once: place by total degree, compute the
    # split, re-place blocks with (lo,hi)-aware LPT.
    local_r = np.empty(N, dtype=np.int64)
    perm_order = np.empty((C, NS), dtype=np.int64)

    def place(lo_d, hi_d, chunk_lock=None):
        """(lo,hi)-aware LPT into blocks. chunk_lock: per-node 0/1 keeps the
        node inside its current allgather chunk so source rows (and therefore
        every edge's lo/hi class) do not move."""
        for c in range(C):
            ids = np.where(core_of == c)[0]
            tot = lo_d[ids] + hi_d[ids]
            order = np.argsort(-tot, kind="stable")
            loads = np.zeros((NBLK, 2))
            used = np.zeros(NBLK, np.int64)
            assign = [[] for _ in range(NBLK)]
            for j in order:
                oid = ids[j]
                if chunk_lock is None:
                    cand = range(NBLK)
                elif chunk_lock[oid] == 0:
                    cand = range(CH0B)
                else:
                    cand = range(CH0B, NBLK)
                best, bestv = -1, None
                for bi in cand:
                    if used[bi] >= CAPS[bi]:
                        continue
                    v = max(loads[bi, 0] + lo_d[oid], loads[bi, 1] + hi_d[oid])
                    if bestv is None or v < bestv:
                        best, bestv = bi, v
                assign[best].append(oid)
                loads[best, 0] += lo_d[oid]
                loads[best, 1] += hi_d[oid]
                used[best] += 1
            pos = 0
            for bi in range(NBLK):
                for oid in assign[bi]:
                    local_r[oid] = pos
                    perm_order[c, pos] = oid
                    pos += 1

    # pass 1: split unknown -> balance by total degree
    place(deg.astype(np.float64), np.zeros(N))
    r = local_r
    phys = np.where(r < CH0, core_of * CH0 + r,
                    HALF + core_of * CH1 + (r - CH0))
    # actual lo/hi degree per dst node under this placement
    hi_e = (phys[src] >= HALF).astype(np.int64)
    lo_deg = np.bincount(dst, weights=1 - hi_e, minlength=N).astype(np.int64)
    hi_deg = np.bincount(dst, weights=hi_e, minlength=N).astype(np.int64)
    lock = (local_r >= CH0).astype(np.int64)

    # pass 2: quota packing. Per chunk, give each block a slot quota that is a
    # multiple of 128 sized to the worst core, then best-fit-decreasing within
    # the quotas; counts land just under tile boundaries instead of just over.
    chunk_rng = [range(CH0B), range(CH0B, NBLK)]
    for c in range(C):
        for ch in (0, 1):
            ids = np.where((core_of == c) & (lock == ch))[0]
            blocks = list(chunk_rng[ch])
            nb = len(blocks)
            # worst-core totals for this chunk decide the quotas (uniform T)
            mask = lock == ch
            lo_tot = max(lo_deg[(core_of == cc) & mask].sum() for cc in range(C))
            hi_tot = max(hi_deg[(core_of == cc) & mask].sum() for cc in range(C))
            tl = int(-(-lo_tot // 128)) + 2
            th = int(-(-hi_tot // 128)) + 2
            base_l, ext_l = divmod(tl, nb)
            base_h, ext_h = divmod(th, nb)
            q_lo = np.array([(base_l + (i < ext_l)) * 128 for i in range(nb)],
                            dtype=np.int64)
            q_hi = np.array([(base_h + (i < ext_h)) * 128 for i in range(nb)],
                            dtype=np.int64)
            caps = np.array([CAPS[b] for b in blocks], dtype=np.int64)
            loads = np.zeros((nb, 2), dtype=np.int64)
            used = np.zeros(nb, dtype=np.int64)
            assign = [[] for _ in range(nb)]
            order = np.argsort(-(lo_deg[ids] + hi_deg[ids]), kind="stable")
            for j in order:
                oid = ids[j]
                ld, hd = lo_deg[oid], hi_deg[oid]
                best, bestv = -1, None
                # feasible blocks: keep all blocks growing evenly toward their
                # quotas (min fill ratio after placement)
                for i in range(nb):
                    if used[i] >= caps[i]:
                        continue
                    if loads[i, 0] + ld > q_lo[i] or loads[i, 1] + hd > q_hi[i]:
                        continue
                    v = max((loads[i, 0] + ld) / q_lo[i],
                            (loads[i, 1] + hd) / q_hi[i])
                    # discourage starving node slots: blocks must end full
                    v += 0.002 * (128 - (caps[i] - used[i]))
                    if bestv is None or v < bestv:
                        best, bestv = i, v
                if best < 0:
                    # repair: max min-slack block with node room
                    for i in range(nb):
                        if used[i] >= caps[i]:
                            continue
                        v = min(q_lo[i] - loads[i, 0] - ld,
                                q_hi[i] - loads[i, 1] - hd)
                        if bestv is None or v > bestv:
                            best, bestv = i, v
                assign[best].append(oid)
                loads[best, 0] += ld
                loads[best, 1] += hd
                used[best] += 1
            for i, b in enumerate(blocks):
                pos = b * 128
                for oid in assign[i]:
                    local_r[oid] = pos
                    perm_order[c, pos] = oid
                    pos += 1

    # physical (chunked) row layout for the gather tables
    r = local_r
    phys = np.where(r < CH0, core_of * CH0 + r,
                    HALF + core_of * CH1 + (r - CH0))

    src_p = phys[src]
    dcore = core_of[dst]
    dloc = local_r[dst]
    blk = dloc // 128
    dcol = dloc % 128
    halfv = (src_p >= HALF).astype(np.int64)
    key = (dcore * NBLK + blk) * 2 + halfv
    eorder = np.argsort(key, kind="stable")
    counts = np.bincount(key, minlength=C * NBLK * 2).reshape(C, NBLK, 2)

    T_lo = -(-counts[:, :, 0].max(axis=0) // 128)  # ceil, uniform across cores
    T_hi = -(-counts[:, :, 1].max(axis=0) // 128)
    T_all = T_lo + T_hi
    TT = int(T_all.sum())
    toff = np.zeros(NBLK, np.int64)
    toff[1:] = np.cumsum(T_all)[:-1]

    src_sorted = src_p[eorder]
    dcol_sorted = dcol[eorder]
    starts = np.zeros(C * NBLK * 2 + 1, np.int64)
    starts[1:] = np.cumsum(counts.reshape(-1))

    idx_all = np.zeros((C, TT * 128), np.int16)           # pad -> row 0
    dc_all = np.full((C, TT * 128), -1, np.int64)         # pad -> -1
    for c in range(C):
        for b in range(NBLK):
            for h in range(2):
                k = (c * NBLK + b) * 2 + h
                s0, s1 = starts[k], starts[k + 1]
                n = s1 - s0
                if n == 0:
                    continue
                slot0 = (toff[b] + (T_lo[b] if h else 0)) * 128
                seg = src_sorted[s0:s1]
                if h:
                    seg = seg - HALF
                idx_all[c, slot0:slot0 + n] = seg.astype(np.int16)
                dc_all[c, slot0:slot0 + n] = dcol_sorted[s0:s1]

    # one-hot scatter matrices in fp8 (exact 0.0 / 1.0)
    slots = np.arange(TT * 128)
    t_of = slots // 128
    ep_of = slots % 128
    ONE8 = np.float32(1.0).astype(FP8NP).view(np.uint8)  # 0x38

    # weights: fold attention projections in; feat-major (f,h) column order
    W1_64 = np.asarray(W1, np.float64)
    Dsrc1 = np.zeros((H1 * HID, H1), np.float64)
    Ddst1 = np.zeros((H1 * HID, H1), np.float64)
    for h in range(H1):
        Dsrc1[h * HID:(h + 1) * HID, h] = np.asarray(a_src1, np.float64)[h]
        Ddst1[h * HID:(h + 1) * HID, h] = np.asarray(a_dst1, np.float64)[h]
    # feat-major index: new position f*4+h  <- old position h*128+f
    ff = np.arange(512)
    fmaj = (ff % 4) * 128 + ff // 4   # old index stored at new slot ff
    rhs1 = np.concatenate(
        [W1_64[:, fmaj], W1_64 @ Dsrc1, W1_64 @ Ddst1], axis=1).astype(BF16NP)
    W2_64 = np.asarray(W2, np.float64)  # rows indexed by old h*128+f order
    rhs2 = np.concatenate(
        [W2_64[fmaj, :],
         (W2_64 @ np.asarray(a_dst2, np.float64)[0][:, None])[fmaj, :]],
        axis=1).astype(BF16NP)                            # [512 (f-major), 129]

    b1_fm = np.asarray(b1, np.float64)[fmaj]
    b1_bc = np.tile(b1_fm[None, :], (128, 1)).astype(BF16NP)
    b2_bc = np.tile(np.asarray(b2, np.float32)[None, :], (128, 1)).astype(BF16NP)
    asrc2_bc = np.tile(np.asarray(a_src2, np.float32)[0][None, :],
                       (128, 1)).astype(BF16NP)
    c2 = float(np.asarray(b2, np.float64) @ np.asarray(a_src2, np.float64)[0])
    ident = np.eye(128, dtype=np.float32)

    xnp = np.asarray(x, np.float32)
    in_maps = []
    for c in range(C):
        xT = np.ascontiguousarray(xnp[perm_order[c]].T).astype(BF16NP)  # [512, NS]
        idx_w = np.ascontiguousarray(
            np.tile(idx_all[c].reshape(-1, 16).T, (8, 1)))  # [128, TT*8]
        dc = dc_all[c]
        valid = dc >= 0
        S8 = np.zeros((128, TT * 128), np.uint8)
        S8[ep_of[valid], t_of[valid] * 128 + dc[valid]] = ONE8
        ST8 = np.zeros((128, TT * 128), np.uint8)
        ST8[dc[valid], t_of[valid] * 128 + ep_of[valid]] = ONE8
        in_maps.append({
            "xT": xT, "rhs1": rhs1, "rhs2": rhs2,
            "b1bc": b1_bc, "b2bc": b2_bc, "asrc2bc": asrc2_bc,
            "identf": ident,
            "idx": idx_w,
            "S8": S8.view(FP8NP), "ST8": ST8.view(FP8NP),
        })

    meta = {
        "T_lo": [int(v) for v in T_lo],
        "T_hi": [int(v) for v in T_hi],
        "toff": [int(v) for v in toff],
        "TT": TT,
        "c2": c2,
    }
    return in_maps, meta, perm_order


# --------------------------------------------------------------------------
# device program
# --------------------------------------------------------------------------

def _build(meta):
    nc = bacc.Bacc("TRN2", target_bir_lowering=False, debug=False, num_devices=C)
    TT = meta["TT"]

    xT_d = nc.dram_tensor("xT", [F_IN, NS], BF16, kind="ExternalInput").ap()
    rhs1_d = nc.dram_tensor("rhs1", [F_IN, 520], BF16, kind="ExternalInput").ap()
    rhs2_d = nc.dram_tensor("rhs2", [F_IN, 129], BF16, kind="ExternalInput").ap()
    b1bc_d = nc.dram_tensor("b1bc", [128, 512], BF16, kind="ExternalInput").ap()
    b2bc_d = nc.dram_tensor("b2bc", [128, 128], BF16, kind="ExternalInput").ap()
    asrc2bc_d = nc.dram_tensor("asrc2bc", [128, 128], BF16, kind="ExternalInput").ap()
    identf_d = nc.dram_tensor("identf", [128, 128], F32, kind="ExternalInput").ap()
    idx_d = nc.dram_tensor("idx", [128, TT * 8], I16, kind="ExternalInput").ap()
    S8_d = nc.dram_tensor("S8", [128, TT * 128], FP8, kind="ExternalInput").ap()
    ST8_d = nc.dram_tensor("ST8", [128, TT * 128], FP8, kind="ExternalInput").ap()
    out_d = nc.dram_tensor("out", [NS, HID], F32, kind="ExternalOutput").ap()

    groups = [list(range(C))]

    with tile.TileContext(nc, num_cores=C) as tc:
        with tc.tile_pool(name="dram", bufs=1, space="DRAM") as dram:
            hb1 = dram.tile([NS, ROW1], BF16)
            hfull1a = dram.tile([HALF, ROW1], BF16, addr_space="Shared")
            hfull1b = dram.tile([N - HALF, ROW1], BF16, addr_space="Shared")
            hb2 = dram.tile([NS, ROW2], BF16)
            hfull2a = dram.tile([HALF, ROW2], BF16, addr_space="Shared")
            hfull2b = dram.tile([N - HALF, ROW2], BF16, addr_space="Shared")

            with tc.tile_pool(name="lv", bufs=1) as lv:
                rhs1_sb = []
                rhs2_sb = []
                for k in range(4):
                    rt = lv.tile([128, 520], BF16, name=f"rhs1sb{k}")
                    nc.sync.dma_start(rt[:], rhs1_d[k * 128:(k + 1) * 128, :])
                    rhs1_sb.append(rt)
                    rt2 = lv.tile([128, 129], BF16, name=f"rhs2sb{k}")
                    nc.sync.dma_start(rt2[:], rhs2_d[k * 128:(k + 1) * 128, :])
                    rhs2_sb.append(rt2)
                b1bc_sb = lv.tile([128, 512], BF16, name="b1bc")
                nc.sync.dma_start(b1bc_sb[:], b1bc_d)
                b2bc_sb = lv.tile([128, 128], BF16, name="b2bc")
                nc.sync.dma_start(b2bc_sb[:], b2bc_d)
                asrc2bc_sb = lv.tile([128, 128], BF16, name="asrc2bc")
                nc.sync.dma_start(asrc2bc_sb[:], asrc2bc_d)
                identf_sb = lv.tile([128, 128], F32, name="identf")
                nc.sync.dma_start(identf_sb[:], identf_d)
                adst1_all = lv.tile([128, NBLK * 8], BF16, name="adst1")
                adst2_all = lv.tile([128, NBLK * 2], BF16, name="adst2")
                pself1_all = lv.tile([128, NBLK * 4], F32, name="pself1")
                pself2_all = lv.tile([128, NBLK], F32, name="pself2")

                # ---------------- phase A: h1 shard + attn scalars ------------
                with (
                    tc.tile_pool(name="a_w", bufs=4) as sbw,
                    tc.tile_pool(name="a_p", bufs=3, space="PSUM") as psp,
                ):
                    for b in range(NBLK):
                        bs = CAPS[b]
                        base = b * 128
                        ph = psp.tile([128, 512], F32, tag="ph")
                        pa = psp.tile([128, 8], F32, tag="pa")
                        xt = sbw.tile([128, 4, 128], BF16, tag="xt")
                        nc.sync.dma_start(
                            xt[:, :, :bs],
                            xT_d[:, base:base + bs].rearrange(
                                "(k p) c -> p k c", k=4))
                        for k in range(4):
                            nc.tensor.matmul(ph[:bs, :], xt[:, k, :bs],
                                             rhs1_sb[k][:, 0:512],
                                             start=(k == 0), stop=(k == 3))
                            nc.tensor.matmul(pa[:bs, :], xt[:, k, :bs],
                                             rhs1_sb[k][:, 512:520],
                                             start=(k == 0), stop=(k == 3))
                        ha = sbw.tile([128, 512], BF16, tag="ha")
                        nc.vector.tensor_tensor(ha[:bs, :], ph[:bs, :],
                                                b1bc_sb[:bs, :], OP.add)
                        asx = sbw.tile([128, 4], F32, tag="asx")
                        nc.scalar.activation(asx[:bs], pa[:bs, 0:4], AF.Copy)
                        adh = adst1_all[:, b * 8:b * 8 + 4]
                        adl = adst1_all[:, b * 8 + 4:b * 8 + 8]
                        if bs < 128:
                            nc.vector.memset(adst1_all[:, b * 8:(b + 1) * 8], 0.0)
                        nc.scalar.activation(adh[:bs], pa[:bs, 4:8], AF.Copy)
                        nc.vector.tensor_tensor(adl[:bs], pa[:bs, 4:8], adh[:bs],
                                                OP.subtract)
                        # self-loop logits: p = exp(leaky(asrc_i + adst_i))
                        eps_ = sbw.tile([128, 4], F32, tag="eps_")
                        nc.vector.tensor_tensor(eps_[:bs], asx[:bs],
                                                pa[:bs, 4:8], OP.add)
                        lrs = sbw.tile([128, 4], F32, tag="lrs")
                        nc.vector.scalar_tensor_tensor(lrs[:bs], eps_[:bs], NEG,
                                                       eps_[:bs], OP.mult, OP.max)
                        if bs < 128:
                            nc.vector.memset(pself1_all[:, b * 4:(b + 1) * 4], 0.0)
                        nc.scalar.activation(pself1_all[:bs, b * 4:(b + 1) * 4],
                                             lrs[:bs], AF.Exp)
                        nc.sync.dma_start(hb1[base:base + bs, 0:512], ha[:bs, :])
                        nc.sync.dma_start(hb1[base:base + bs, 512:520],
                                          asx[:bs, :].bitcast(BF16))
                        if b == CH0B - 1:
                            nc.gpsimd.collective_compute(
                                "AllGather", OP.bypass, replica_groups=groups,
                                ins=[hb1[0:CH0, :].opt()],
                                outs=[hfull1a[:].opt()])
                nc.gpsimd.collective_compute(
                    "AllGather", OP.bypass, replica_groups=groups,
                    ins=[hb1[CH0:NS, :].opt()], outs=[hfull1b[:].opt()])

                # ---------------- phase B: layer-1 edge pass + h2@W2 ----------
                with (
                    tc.tile_pool(name="b_m", bufs=3) as sbm,
                    tc.tile_pool(name="b_g", bufs=2) as sbg,
                    tc.tile_pool(name="b_s", bufs=2) as sbs,
                    tc.tile_pool(name="b_w", bufs=2) as sbw2,
                    tc.tile_pool(name="b_pb", bufs=2, space="PSUM") as psb,
                    tc.tile_pool(name="b_pm", bufs=2, space="PSUM") as psm,
                    tc.tile_pool(name="b_ph", bufs=1, space="PSUM") as psh,
                ):
                    for b in range(NBLK):
                        bs = CAPS[b]
                        base = b * 128
                        T_lo, T_hi = meta["T_lo"][b], meta["T_hi"][b]
                        T = T_lo + T_hi
                        boff = meta["toff"][b]

                        idx_sb = sbm.tile([128, T * 8], I16, tag="idx")
                        nc.sync.dma_start(idx_sb[:], idx_d[:, boff * 8:(boff + T) * 8])
                        S8_sb = sbm.tile([128, T * 128], FP8, tag="S8")
                        nc.sync.dma_start(S8_sb[:], S8_d[:, boff * 128:(boff + T) * 128])
                        ST8_sb = sbm.tile([128, T * 128], FP8, tag="ST8")
                        nc.sync.dma_start(ST8_sb[:], ST8_d[:, boff * 128:(boff + T) * 128])

                        gat = sbg.tile([128, T, ROW1], BF16, tag="gat")
                        if T_lo:
                            nc.gpsimd.dma_gather(
                                gat[:, 0:T_lo, :], hfull1a[:],
                                idx_sb[:, 0:T_lo * 8], T_lo * 128, T_lo * 128,
                                ROW1, elem_step=ROW1, single_packet=False)
                        if T_hi:
                            nc.gpsimd.dma_gather(
                                gat[:, T_lo:T, :], hfull1b[:],
                                idx_sb[:, T_lo * 8:T * 8], T_hi * 128, T_hi * 128,
                                ROW1, elem_step=ROW1, single_packet=False)

                        ae = psm.tile([128, T * 8], F32, tag="ae")
                        adst_blk = adst1_all[:, b * 8:(b + 1) * 8]
                        for t in range(T):
                            nc.tensor.matmul(ae[:, t * 8:(t + 1) * 8],
                                             ST8_sb[:, t * 128:(t + 1) * 128],
                                             adst_blk, start=True, stop=True)

                        asrc_v = gat[:, :, 512:520].bitcast(F32)      # [128,T,4]
                        ae3 = ae[:].rearrange("p (t x) -> p t x", x=8)
                        ep1 = sbs.tile([128, T * 4], F32, tag="ep1")
                        nc.vector.tensor_tensor(
                            ep1[:].rearrange("p (t h) -> p t h", h=4),
                            asrc_v, ae3[:, :, 0:4], OP.add)
                        ep2 = sbs.tile([128, T * 4], F32, tag="ep2")
                        nc.vector.tensor_tensor(
                            ep2[:].rearrange("p (t h) -> p t h", h=4),
                            ep1[:].rearrange("p (t h) -> p t h", h=4),
                            ae3[:, :, 4:8], OP.add)
                        lr = sbs.tile([128, T * 4], F32, tag="lr")
                        nc.vector.scalar_tensor_tensor(lr[:], ep2[:], NEG, ep2[:],
                                                       OP.mult, OP.max)
                        p_all = sbs.tile([128, T * 4], BF16, tag="p")
                        nc.scalar.activation(p_all[:], lr[:], AF.Exp)

                        # w[e, t, f, h] = h[e, t, f, h] * p[e, t, h] (feat-major)
                        w_all = sbg.tile([128, T * 512], BF16, tag="w")
                        nc.vector.tensor_tensor(
                            w_all[:].rearrange("p (t f h) -> p t f h", t=T, h=4),
                            gat[:, :, 0:512].rearrange("p t (f h) -> p t f h", h=4),
                            p_all[:].rearrange("p (t h) -> p t h", t=T)
                                .unsqueeze(2).broadcast_to([128, T, 128, 4]),
                            OP.mult)

                        oacc = psb.tile([128, 512], F32, tag="oacc")
                        dacc = psb.tile([128, 4], F32, tag="dacc")
                        for t in range(T):
                            nc.tensor.matmul(oacc[:], S8_sb[:, t * 128:(t + 1) * 128],
                                             w_all[:, t * 512:(t + 1) * 512],
                                             start=(t == 0), stop=(t == T - 1))
                            nc.tensor.matmul(dacc[:], S8_sb[:, t * 128:(t + 1) * 128],
                                             p_all[:, t * 4:(t + 1) * 4],
                                             start=(t == 0), stop=(t == T - 1))

                        # block writer: self-loop add, normalize, ELU, h2 @ rhs2
                        hself = sbw2.tile([128, 512], BF16, tag="hself")
                        nc.sync.dma_start(hself[:bs, :], hb1[base:base + bs, 0:512])
                        selfw = sbw2.tile([128, 512], F32, tag="selfw")
                        nc.vector.tensor_tensor(
                            selfw[:].rearrange("p (f h) -> p f h", h=4),
                            hself[:].rearrange("p (f h) -> p f h", h=4),
                            pself1_all[:, b * 4:(b + 1) * 4]
                                .unsqueeze(1).broadcast_to([128, 128, 4]),
                            OP.mult)
                        osum = sbw2.tile([128, 512], F32, tag="osum")
                        nc.vector.tensor_tensor(osum[:], oacc[:], selfw[:], OP.add)
                        den = sbs.tile([128, 4], F32, tag="den")
                        nc.vector.scalar_tensor_tensor(
                            den[:], dacc[:], EPS,
                            pself1_all[:, b * 4:(b + 1) * 4], OP.add, OP.add)
                        rec = sbs.tile([128, 4], F32, tag="rec")
                        nc.vector.reciprocal(rec[:], den[:])
                        h2 = sbw2.tile([128, 512], F32, tag="h2")
                        nc.vector.tensor_tensor(
                            h2[:].rearrange("p (f h) -> p f h", h=4),
                            osum[:].rearrange("p (f h) -> p f h", h=4),
                            rec[:].unsqueeze(1).broadcast_to([128, 128, 4]),
                            OP.mult)
                        rl = sbw2.tile([128, 512], F32, tag="rl")
                        nc.scalar.activation(rl[:], h2[:], AF.Relu)
                        mn = sbw2.tile([128, 512], F32, tag="mn")
                        nc.vector.tensor_scalar_min(mn[:], h2[:], 0.0)
                        em = sbw2.tile([128, 512], F32, tag="em")
                        nc.scalar.activation(em[:], mn[:], AF.Exp)
                        h2f = sbw2.tile([128, 512], F32, tag="h2f")
                        nc.vector.scalar_tensor_tensor(h2f[:], em[:], -1.0, rl[:],
                                                       OP.add, OP.add)
                        hh = psh.tile([128, 129], F32, tag="hh")
                        for k in range(4):
                            tp = psm.tile([128, 128], F32, tag="tp", bufs=1)
                            nc.tensor.transpose(tp[:], h2f[:, k * 128:(k + 1) * 128],
                                                identf_sb[:])
                            h2T = sbs.tile([128, 128], BF16, tag="h2T")
                            nc.scalar.activation(h2T[:], tp[:], AF.Copy)
                            nc.tensor.matmul(hh[:], h2T[:], rhs2_sb[k][:],
                                             start=(k == 0), stop=(k == 3))
                        ha2 = sbw2.tile([128, 128], BF16, tag="ha2")
                        nc.vector.tensor_tensor(ha2[:bs, :], hh[:bs, 0:128],
                                                b2bc_sb[:bs, :], OP.add)
                        ad2h = adst2_all[:, b * 2:b * 2 + 1]
                        ad2l = adst2_all[:, b * 2 + 1:b * 2 + 2]
                        if bs < 128:
                            nc.vector.memset(adst2_all[:, b * 2:(b + 1) * 2], 0.0)
                        nc.scalar.activation(ad2h[:bs], hh[:bs, 128:129], AF.Copy)
                        nc.vector.tensor_tensor(ad2l[:bs], hh[:bs, 128:129],
                                                ad2h[:bs], OP.subtract)
                        # self-loop p2 = exp(leaky(asrc2_i + adst2_i))
                        tt2 = sbw2.tile([128, 128], BF16, tag="tt2")
                        nc.vector.tensor_tensor(tt2[:], ha2[:], asrc2bc_sb[:],
                                                OP.mult)
                        as2 = sbw2.tile([128, 1], F32, tag="as2")
                        nc.vector.tensor_reduce(as2[:], tt2[:],
                                                mybir.AxisListType.X, OP.add)
                        ep2s = sbw2.tile([128, 1], F32, tag="ep2s")
                        nc.vector.scalar_tensor_tensor(ep2s[:bs], as2[:bs],
                                                       -meta["c2"],
                                                       hh[:bs, 128:129],
                                                       OP.add, OP.add)
                        lr2s = sbw2.tile([128, 1], F32, tag="lr2s")
                        nc.vector.scalar_tensor_tensor(lr2s[:bs], ep2s[:bs], NEG,
                                                       ep2s[:bs], OP.mult, OP.max)
                        if bs < 128:
                            nc.vector.memset(pself2_all[:, b:b + 1], 0.0)
                        nc.scalar.activation(pself2_all[:bs, b:b + 1], lr2s[:bs],
                                             AF.Exp)
                        nc.sync.dma_start(hb2[base:base + bs, :], ha2[:bs, :])
                        if b == CH0B - 1:
                            nc.gpsimd.collective_compute(
                                "AllGather", OP.bypass, replica_groups=groups,
                                ins=[hb2[0:CH0, :].opt()],
                                outs=[hfull2a[:].opt()])
                nc.gpsimd.collective_compute(
                    "AllGather", OP.bypass, replica_groups=groups,
                    ins=[hb2[CH0:NS, :].opt()], outs=[hfull2b[:].opt()])

                # ---------------- phase D: layer-2 edge pass ------------------
                with (
                    tc.tile_pool(name="d_m", bufs=3) as sbm,
                    tc.tile_pool(name="d_g", bufs=3) as sbg,
                    tc.tile_pool(name="d_s", bufs=3) as sbs,
                    tc.tile_pool(name="d_pb", bufs=2, space="PSUM") as psb,
                    tc.tile_pool(name="d_pm", bufs=2, space="PSUM") as psm,
                ):
                    for b in range(NBLK):
                        bs = CAPS[b]
                        base = b * 128
                        T_lo, T_hi = meta["T_lo"][b], meta["T_hi"][b]
                        T = T_lo + T_hi
                        boff = meta["toff"][b]

                        idx_sb = sbm.tile([128, T * 8], I16, tag="idx")
                        nc.sync.dma_start(idx_sb[:], idx_d[:, boff * 8:(boff + T) * 8])
                        S8_sb = sbm.tile([128, T * 128], FP8, tag="S8")
                        nc.sync.dma_start(S8_sb[:], S8_d[:, boff * 128:(boff + T) * 128])
                        ST8_sb = sbm.tile([128, T * 128], FP8, tag="ST8")
                        nc.sync.dma_start(ST8_sb[:], ST8_d[:, boff * 128:(boff + T) * 128])

                        gat = sbg.tile([128, T, ROW2], BF16, tag="gat")
                        if T_lo:
                            nc.gpsimd.dma_gather(
                                gat[:, 0:T_lo, :], hfull2a[:],
                                idx_sb[:, 0:T_lo * 8], T_lo * 128, T_lo * 128,
                                ROW2, elem_step=ROW2, single_packet=False)
                        if T_hi:
                            nc.gpsimd.dma_gather(
                                gat[:, T_lo:T, :], hfull2b[:],
                                idx_sb[:, T_lo * 8:T * 8], T_hi * 128, T_hi * 128,
                                ROW2, elem_step=ROW2, single_packet=False)

                        tmp = sbg.tile([128, T * 128], BF16, tag="tmp")
                        nc.vector.tensor_tensor(
                            tmp[:].rearrange("p (t f) -> p t f", t=T),
                            gat[:],
                            asrc2bc_sb[:].unsqueeze(1).broadcast_to([128, T, 128]),
                            OP.mult)
                        asr = sbs.tile([128, T], F32, tag="asr")
                        nc.vector.tensor_reduce(
                            asr[:], tmp[:].rearrange("p (t f) -> p t f", t=T),
                            mybir.AxisListType.X, OP.add)

                        ae = psm.tile([128, T * 2], F32, tag="ae")
                        adst_blk = adst2_all[:, b * 2:(b + 1) * 2]
                        for t in range(T):
                            nc.tensor.matmul(ae[:, t * 2:(t + 1) * 2],
                                             ST8_sb[:, t * 128:(t + 1) * 128],
                                             adst_blk, start=True, stop=True)

                        ae3 = ae[:].rearrange("p (t x) -> p t x", x=2)
                        ep1 = sbs.tile([128, T], F32, tag="ep1")
                        nc.vector.scalar_tensor_tensor(
                            ep1[:].unsqueeze(2), asr[:].unsqueeze(2), -meta["c2"],
                            ae3[:, :, 0:1], OP.add, OP.add)
                        ep2 = sbs.tile([128, T], F32, tag="ep2")
                        nc.vector.tensor_tensor(ep2[:].unsqueeze(2),
                                                ep1[:].unsqueeze(2),
                                                ae3[:, :, 1:2], OP.add)
                        lr = sbs.tile([128, T], F32, tag="lr")
                        nc.vector.scalar_tensor_tensor(lr[:], ep2[:], NEG, ep2[:],
                                                       OP.mult, OP.max)
                        p_all = sbs.tile([128, T], BF16, tag="p")
                        nc.scalar.activation(p_all[:], lr[:], AF.Exp)
                        # pair-duplicate p so the w multiply has a packed last dim
                        p_dup = sbs.tile([128, T * 2], BF16, tag="pdup")
                        nc.vector.tensor_copy(
                            p_dup[:].rearrange("p (t k) -> p t k", k=2),
                            p_all[:].unsqueeze(2).broadcast_to([128, T, 2]))

                        w_all = sbg.tile([128, T * 128], BF16, tag="w")
                        nc.vector.tensor_tensor(
                            w_all[:].rearrange("p (t j k) -> p t j k", t=T, k=2),
                            gat[:].rearrange("p t (j k) -> p t j k", k=2),
                            p_dup[:].rearrange("p (t k) -> p t k", t=T)
                                .unsqueeze(2).broadcast_to([128, T, 64, 2]),
                            OP.mult)

                        oacc = psb.tile([128, 128], F32, tag="oacc")
                        dacc = psb.tile([128, 1], F32, tag="dacc")
                        for t in range(T):
                            nc.tensor.matmul(oacc[:], S8_sb[:, t * 128:(t + 1) * 128],
                                             w_all[:, t * 128:(t + 1) * 128],
                                             start=(t == 0), stop=(t == T - 1))
                            nc.tensor.matmul(dacc[:], S8_sb[:, t * 128:(t + 1) * 128],
                                             p_all[:, t:t + 1],
                                             start=(t == 0), stop=(t == T - 1))

                        h2self = sbs.tile([128, 128], BF16, tag="h2self")
                        nc.sync.dma_start(h2self[:bs, :], hb2[base:base + bs, :])
                        selfw2 = sbs.tile([128, 128], F32, tag="selfw2")
                        nc.vector.tensor_single_scalar(
                            selfw2[:], h2self[:], pself2_all[:, b:b + 1], OP.mult)
                        osum2 = sbs.tile([128, 128], F32, tag="osum2")
                        nc.vector.tensor_tensor(osum2[:], oacc[:], selfw2[:], OP.add)
                        den = sbs.tile([128, 1], F32, tag="den")
                        nc.vector.scalar_tensor_tensor(
                            den[:], dacc[:], EPS, pself2_all[:, b:b + 1],
                            OP.add, OP.add)
                        rec = sbs.tile([128, 1], F32, tag="rec")
                        nc.vector.reciprocal(rec[:], den[:])
                        of = sbs.tile([128, 128], F32, tag="of")
                        nc.scalar.activation(of[:], osum2[:], AF.Copy,
                                             scale=rec[:, 0:1])
                        nc.sync.dma_start(out_d[base:base + bs, :], of[:bs, :])

    nc.compile()
    return nc


# --------------------------------------------------------------------------
# entry point
# --------------------------------------------------------------------------

def kernel(x, edge_index, W1, a_src1, a_dst1, b1, W2, a_src2, a_dst2, b2,
           _trace=False):
    in_maps, meta, perm_order = _prepare(
        x, edge_index, W1, a_src1, a_dst1, b1, W2, a_src2, a_dst2, b2)

    import time as _time
    _t0 = _time.time()
    key = (meta["TT"], tuple(meta["T_lo"]), tuple(meta["T_hi"]))
    if key not in _cache:
        _cache.clear()
        _cache[key] = _build(meta)
    nc = _cache[key]
    print(f"[kernel] build done at {_time.time()-_t0:.1f}s", flush=True)

    kw = {}
    if _trace:
        kw = dict(trace=True)
    res = bass_utils.run_bass_kernel_spmd(nc, in_maps, core_ids=list(range(C)), **kw)

    out = np.empty((N, HID), np.float32)
    for c in range(C):
        out[perm_order[c]] = res.results[c]["out"]
    kernel._last_result = res
    return out
